# revision 9
# baseline (speedup 1.0000x reference)
"""Trainium2 Bass kernel for nn_ConvNorm4ContPit: 3x (conv1d -> GroupNorm -> ReLU
-> random_resample) over two streams (cont 512ch, pit 128ch), B=32, T=1024.

Sharding: pure data parallelism, 4 examples per core across 8 cores.
Activations are channel-major [C, T] in SBUF. Conv = weight-stationary fp32r
matmuls accumulating over (ci, k) in PSUM. GroupNorm stats via bn_stats on the
conv PSUM + a block-diagonal ones matmul (partition-group reduce+broadcast),
applied fused with ReLU on ACT (per-partition scale/bias). random_resample is
an exact linear op: out = x @ A with A built on the host (<=2 nnz/column from
the gather+lerp), executed as banded fp32r matmuls on PE after transposing x
tiles (exact fp32 PE transposes).
"""
import numpy as np

import concourse.bacc as bacc_mod
import concourse.bass as bass
import concourse.mybir as mybir
import concourse.tile as tile
from concourse.bass_utils import run_bass_kernel_spmd

F32 = mybir.dt.float32
R32 = mybir.dt.float32r

B, T = 32, 1024
CONT_DIM, CONT_H = 80, 512
PIT_DIM, PIT_H = 4, 128
LAYERS = 3
MAX_PAD = 1024
MIN_SEG, MAX_SEG = 19, 32
NUM_SEG = MAX_PAD // MIN_SEG + 1   # 54
SEG_L = MAX_SEG * 2                # 64
EPS = 1e-5
NCORES = 8
BPC = B // NCORES                  # 4 examples per core
CH = CONT_H + PIT_H                # 640
NT = T // 128                      # 8 time tiles
TPAD = T + 4                       # 'same' pad 2 each side
CHUNK = 256                        # resample t_out chunk (l0/l1); >=256 for fp32r speed
NCHUNK = T // CHUNK                # 4

# tunables (overridden by experiments)
CFG = dict(
    cv_shape=512, cv_bufs=5,       # conv psum tile free width / bufs
    rs_bufs=1,                     # resample psum bufs ([128,512] each)
    tp_bufs=2,                     # transpose psum bufs ([128,512] each)
    rs_evict="act",                # engine for resample evict: act|vector
    conv_pair=False,               # interleave conv halves per weight
    tiny_engine="vector",          # engine for small stats TT ops
)


# ----------------------------------------------------------------------------
# Host-side: exact replication of the reference resample indexing -> A matrices
# ----------------------------------------------------------------------------
def _resample_matrix(scales_row, len_row):
    """A [T_in=1024, T_out=1024] f32 with A[gi,d]=1-lam, A[gi+1,d]=lam per valid
    candidate; bit-exact f32 replication of reference.random_resample."""
    idx = np.arange(SEG_L, dtype=np.float32)
    sc = scales_row.astype(np.float32).reshape(NUM_SEG, 1)
    idx_scaled = idx[None, :] / sc                       # [S, L] f32 div
    idx_fl = np.floor(idx_scaled)
    lam = (idx_scaled - idx_fl).reshape(-1)
    ln = len_row.astype(np.int64).reshape(NUM_SEG, 1)
    mask1 = idx_fl < (ln - 1).astype(np.float32)
    offset = np.cumsum(len_row.astype(np.int64))
    offset = np.concatenate([[0], offset[:-1]]).reshape(NUM_SEG, 1)
    idx_org = idx_fl.astype(np.int64) + offset
    mask2 = idx_org < (MAX_PAD - 1)
    mask = (mask1 & mask2).reshape(-1)
    mi = mask.astype(np.int64)
    dest = np.cumsum(mi) - mi
    valid = mask & (dest < MAX_PAD)
    gi = np.clip(idx_org.reshape(-1), 0, T - 2)
    A = np.zeros((T, T), dtype=np.float32)
    gv = gi[valid]
    dv = dest[valid]
    lv = lam[valid]
    A[gv, dv] = np.float32(1.0) - lv
    A[gv + 1, dv] = lv
    return A


def _band(A_all, c0, c1):
    """Union tile band [lo, hi) over a list of A matrices for out cols [c0,c1)."""
    lo_t, hi_t = NT, 0
    for A in A_all:
        rows = np.flatnonzero(np.abs(A[:, c0:c1]).sum(axis=1))
        if rows.size == 0:
            continue
        lo_t = min(lo_t, rows[0] // 128)
        hi_t = max(hi_t, rows[-1] // 128 + 1)
    if lo_t >= hi_t:
        lo_t, hi_t = 0, 1
    return int(lo_t), int(hi_t)


def _blockones():
    M = np.zeros((128, 128), dtype=np.float32)
    for g in range(8):
        M[g * 16:(g + 1) * 16, g * 16:(g + 1) * 16] = np.float32(1.0 / 16.0)
    return M


def _prep_host(inputs):
    """Build all per-core input arrays + static band tables."""
    g = {k: np.asarray(v) for k, v in inputs.items()}
    mel = g["mel"].astype(np.float32)        # [B, T, 80]
    f0 = g["f0"].astype(np.float32)          # [B, T, 4]

    # conv weights -> lhsT layout [ci, k, co]
    wc0 = np.ascontiguousarray(g["cont_w0"].transpose(1, 2, 0)).astype(np.float32)  # [80,5,512]
    wp0 = np.ascontiguousarray(g["pit_w0"].transpose(1, 2, 0)).astype(np.float32)   # [4,5,128]
    # layers 1,2: [l, ci_tile, 128, 5, co]
    wc12 = np.ascontiguousarray(
        g["cont_w"].transpose(0, 2, 3, 1).reshape(2, 4, 128, 5, CONT_H)).astype(np.float32)
    wp12 = np.ascontiguousarray(
        g["pit_w"].transpose(0, 2, 3, 1).reshape(2, 1, 128, 5, PIT_H)).astype(np.float32)

    # per-layer packs [L, 128, 5]: ctiles 0..3 = cont co slices, 4 = pit
    def pack(cont, pit):  # cont [L,512], pit [L,128] -> [L,128,5]
        out = np.empty((LAYERS, 128, 5), dtype=np.float32)
        for l in range(LAYERS):
            for j in range(4):
                out[l, :, j] = cont[l, j * 128:(j + 1) * 128]
            out[l, :, 4] = pit[l]
        return out
    bias_pack = pack(np.asarray(g["cont_b"], np.float32), np.asarray(g["pit_b"], np.float32))
    gamma_pack = pack(np.asarray(g["cont_gamma"], np.float32), np.asarray(g["pit_gamma"], np.float32))
    beta_pack = pack(np.asarray(g["cont_beta"], np.float32), np.asarray(g["pit_beta"], np.float32))

    scales = np.asarray(g["scales"], np.float32).reshape(LAYERS, B, NUM_SEG)
    len_seg = np.asarray(g["len_seg"]).reshape(LAYERS, B, NUM_SEG)

    A = [[_resample_matrix(scales[l, b], len_seg[l, b]) for b in range(B)]
         for l in range(LAYERS)]

    # static band tables (global over all B -> identical program on all cores)
    LO = np.zeros((2, NCHUNK), np.int64)
    KT = np.zeros((2, NCHUNK), np.int64)
    for l in range(2):
        for c in range(NCHUNK):
            lo, hi = _band(A[l], c * CHUNK, (c + 1) * CHUNK)
            LO[l, c], KT[l, c] = lo, hi - lo
    LO2 = np.zeros(NT, np.int64)
    KT2 = np.zeros(NT, np.int64)
    for t in range(NT):
        lo, hi = _band(A[2], t * 128, (t + 1) * 128)
        LO2[t], KT2[t] = lo, hi - lo

    # pack banded A blocks per core
    per_core = []
    for core in range(NCORES):
        bs = range(core * BPC, (core + 1) * BPC)
        m = {}
        # mel/f0 channel-major, time-padded by 2 on each side
        mel_cm = np.zeros((BPC, CONT_DIM, TPAD), np.float32)
        f0_cm = np.zeros((BPC, PIT_DIM, TPAD), np.float32)
        for i, b in enumerate(bs):
            mel_cm[i, :, 2:2 + T] = mel[b].T
            f0_cm[i, :, 2:2 + T] = f0[b].T
        m["mel"] = mel_cm
        m["f0"] = f0_cm
        m["wc0"] = wc0
        m["wp0"] = wp0
        m["wc12"] = wc12
        m["wp12"] = wp12
        m["bias_pack"] = bias_pack
        m["gamma_pack"] = gamma_pack
        m["beta_pack"] = beta_pack
        m["blockones"] = _blockones()
        m["zpad"] = np.zeros((128, 2), np.float32)
        m["ident"] = np.eye(128, dtype=np.float32)
        for l in range(2):
            for c in range(NCHUNK):
                kt, lo = int(KT[l, c]), int(LO[l, c])
                blk = np.zeros((BPC, kt, 128, CHUNK), np.float32)
                for i, b in enumerate(bs):
                    blk[i] = A[l][b][lo * 128:(lo + kt) * 128,
                                     c * CHUNK:(c + 1) * CHUNK].reshape(kt, 128, CHUNK)
                m[f"ra{l}c{c}"] = blk
        for t in range(NT):
            kt, lo = int(KT2[t]), int(LO2[t])
            blk = np.zeros((BPC, kt, 128, 128), np.float32)
            for i, b in enumerate(bs):
                blk[i] = A[2][b][lo * 128:(lo + kt) * 128,
                                 t * 128:(t + 1) * 128].reshape(kt, 128, 128)
            m[f"ra2t{t}"] = blk
        per_core.append(m)

    tables = dict(LO=LO, KT=KT, LO2=LO2, KT2=KT2)
    return per_core, tables


# ----------------------------------------------------------------------------
# Device program
# ----------------------------------------------------------------------------
def build_program(tables):
    LO, KT, LO2, KT2 = tables["LO"], tables["KT"], tables["LO2"], tables["KT2"]
    KTMAXW = max(int(KT.max()) * CHUNK, int(KT2.max()) * 128)  # A tile free width

    nc = bacc_mod.Bacc("TRN2", target_bir_lowering=False)
    AF = mybir.ActivationFunctionType

    mel_in = nc.dram_tensor("mel", [BPC, CONT_DIM, TPAD], R32, kind="ExternalInput")
    f0_in = nc.dram_tensor("f0", [BPC, PIT_DIM, TPAD], R32, kind="ExternalInput")
    wc0_in = nc.dram_tensor("wc0", [CONT_DIM, 5, CONT_H], R32, kind="ExternalInput")
    wp0_in = nc.dram_tensor("wp0", [PIT_DIM, 5, PIT_H], R32, kind="ExternalInput")
    wc12_in = nc.dram_tensor("wc12", [2, 4, 128, 5, CONT_H], R32, kind="ExternalInput")
    wp12_in = nc.dram_tensor("wp12", [2, 1, 128, 5, PIT_H], R32, kind="ExternalInput")
    bias_in = nc.dram_tensor("bias_pack", [LAYERS, 128, 5], F32, kind="ExternalInput")
    gamma_in = nc.dram_tensor("gamma_pack", [LAYERS, 128, 5], F32, kind="ExternalInput")
    beta_in = nc.dram_tensor("beta_pack", [LAYERS, 128, 5], F32, kind="ExternalInput")
    bones_in = nc.dram_tensor("blockones", [128, 128], F32, kind="ExternalInput")
    ident_in = nc.dram_tensor("ident", [128, 128], F32, kind="ExternalInput")
    ra = {}
    for l in range(2):
        for c in range(NCHUNK):
            ra[(l, c)] = nc.dram_tensor(f"ra{l}c{c}", [BPC, int(KT[l, c]), 128, CHUNK],
                                        R32, kind="ExternalInput")
    ra2 = {}
    for t in range(NT):
        ra2[t] = nc.dram_tensor(f"ra2t{t}", [BPC, int(KT2[t]), 128, 128],
                                R32, kind="ExternalInput")

    zpad_in = nc.dram_tensor("zpad", [128, 2], R32, kind="ExternalInput")
    mel_out = nc.dram_tensor("mel_out", [BPC, T, CONT_H], F32, kind="ExternalOutput")
    f0_out = nc.dram_tensor("f0_out", [BPC, T, PIT_H], F32, kind="ExternalOutput")

    with tile.TileContext(nc) as tc:
        with (
            tc.tile_pool(name="wpool", bufs=1) as wpool,
            tc.tile_pool(name="xpool", bufs=1) as xpool,
            tc.tile_pool(name="ypool", bufs=1) as ypool,
            tc.tile_pool(name="xtpool", bufs=1) as xtpool,
            tc.tile_pool(name="apool", bufs=2) as apool,
            tc.tile_pool(name="spool", bufs=2) as spool,
            tc.tile_pool(name="cpool", bufs=1) as cpool,
            tc.tile_pool(name="opool", bufs=2) as opool,
            tc.tile_pool(name="cvps", bufs=CFG["cv_bufs"], space="PSUM") as cvps,
            tc.tile_pool(name="rsps", bufs=CFG["rs_bufs"], space="PSUM") as rsps,
            tc.tile_pool(name="tpps", bufs=CFG["tp_bufs"], space="PSUM") as tpps,
        ):
            # ---- constants ----
            bones = cpool.tile([128, 128], F32, tag="bones")
            nc.sync.dma_start(bones, bones_in[:, :])
            ident = cpool.tile([128, 128], F32, tag="ident")
            nc.sync.dma_start(ident, ident_in[:, :])
            eps_t = cpool.tile([128, 1], F32, tag="eps")
            nc.vector.memset(eps_t, EPS)

            # ---- persistent x tiles (conv inputs), zeroed borders ----
            xt_x = {}
            for b in range(BPC):
                for j in range(5):
                    xx = xpool.tile([128, TPAD], R32, tag=f"x_{b}_{j}")
                    xt_x[(b, j)] = xx
            # layer-0 inputs land in x_{b}_0 (mel, 80 parts), x_{b}_1 (f0, 4 parts)
            for b in range(BPC):
                nc.sync.dma_start(xt_x[(b, 0)][:CONT_DIM, :], mel_in[b])
                nc.sync.dma_start(xt_x[(b, 1)][:PIT_DIM, :], f0_in[b])
            # pad-border zeroing (needed from layer-1 on; keep off critical path)
            for b in range(BPC):
                for j in range(5):
                    xx = xt_x[(b, j)]
                    nc.gpsimd.dma_start(xx[:, 0:2], zpad_in[:, :])
                    nc.gpsimd.dma_start(xx[:, T + 2:TPAD], zpad_in[:, :])

            for l in range(LAYERS):
                # ---- load weights for this layer ----
                w_cont = []
                if l == 0:
                    w0 = wpool.tile([128, 5, CONT_H], R32, tag="wc0")
                    nc.sync.dma_start(w0[:CONT_DIM], wc0_in[:, :, :])
                    w_cont.append(w0)
                    wp = wpool.tile([128, 5, PIT_H], R32, tag="wp")
                    nc.sync.dma_start(wp[:PIT_DIM], wp0_in[:, :, :])
                else:
                    for ci_t in range(4):
                        wt = wpool.tile([128, 5, CONT_H], R32, tag=f"wc{ci_t}")
                        nc.sync.dma_start(wt, wc12_in[l - 1, ci_t])
                        w_cont.append(wt)
                    wp = wpool.tile([128, 5, PIT_H], R32, tag="wp")
                    nc.sync.dma_start(wp, wp12_in[l - 1, 0])
                gam = cpool.tile([128, 5], F32, tag="gam")
                nc.sync.dma_start(gam, gamma_in[l])
                bet = cpool.tile([128, 5], F32, tag="bet")
                nc.sync.dma_start(bet, beta_in[l])
                bia = cpool.tile([128, 5], F32, tag="bia")
                nc.sync.dma_start(bia, bias_in[l])

                # conv input tiles per stream
                def conv_srcs(b):
                    if l == 0:
                        return ([(xt_x[(b, 0)], CONT_DIM)], [(xt_x[(b, 1)], PIT_DIM)])
                    return ([(xt_x[(b, j)], 128) for j in range(4)],
                            [(xt_x[(b, 4)], 128)])

                for b in range(BPC):
                    cont_src, pit_src = conv_srcs(b)

                    # prefetch A blocks for this (l, b)
                    a_sb = {}
                    if l < 2:
                        for c in range(NCHUNK):
                            kt = int(KT[l, c])
                            at = apool.tile([128, KTMAXW], R32, tag=f"a{c % 2}")
                            nc.sync.dma_start(
                                at[:, 0:kt * CHUNK].rearrange("p (k n) -> p k n", k=kt),
                                ra[(l, c)][b].rearrange("k p n -> p k n"))
                            a_sb[c] = at
                    else:
                        for t in range(NT):
                            kt = int(KT2[t])
                            at = apool.tile([128, KTMAXW], R32, tag=f"a{t % 2}")
                            nc.sync.dma_start(
                                at[:, 0:kt * 128].rearrange("p (k n) -> p k n", k=kt),
                                ra2[t][b].rearrange("k p n -> p k n"))
                            a_sb[t] = at

                    # ---- conv + GN + ReLU per ctile ----
                    y_tiles = []
                    for j in range(5):
                        is_pit = (j == 4)
                        srcs = pit_src if is_pit else cont_src
                        wlist = [wp] if is_pit else w_cont
                        co0 = 0 if is_pit else j * 128
                        if CFG["cv_shape"] == 1024:
                            pst = cvps.tile([128, 1024], F32, tag="cv")
                            halves = [pst[:, 0:512], pst[:, 512:1024]]
                        else:
                            halves = [cvps.tile([128, 512], F32, tag="cv",
                                                 name=f"cvh{b}_{j}_{h}")
                                      for h in range(2)]
                        nmm = len(srcs) * 5
                        st6 = spool.tile([128, 2, 6], F32, tag="st6")
                        if CFG.get("conv_pair", True):
                            i = 0
                            for si, (xsrc, kdim) in enumerate(srcs):
                                wt = wlist[si if not is_pit else 0] if not is_pit else wp
                                for k in range(5):
                                    for half in range(2):
                                        nc.tensor.matmul(
                                            halves[half],
                                            wt[:kdim, k, co0:co0 + 128],
                                            xsrc[:kdim, half * 512 + k:half * 512 + k + 512],
                                            start=(i == 0), stop=(i == nmm - 1))
                                    i += 1
                            for half in range(2):
                                nc.vector.bn_stats(st6[:, half, :], halves[half])
                        else:
                            for half in range(2):
                                i = 0
                                for si, (xsrc, kdim) in enumerate(srcs):
                                    wt = wlist[si if not is_pit else 0] if not is_pit else wp
                                    for k in range(5):
                                        nc.tensor.matmul(
                                            halves[half],
                                            wt[:kdim, k, co0:co0 + 128],
                                            xsrc[:kdim, half * 512 + k:half * 512 + k + 512],
                                            start=(i == 0), stop=(i == nmm - 1))
                                        i += 1
                                nc.vector.bn_stats(st6[:, half, :], halves[half])
                        mv = spool.tile([128, 3], F32, tag="mv")
                        nc.vector.bn_aggr(mv[:, 0:2], st6)
                        # mean' = mean + bias ; stash mean'^2
                        nc.vector.tensor_add(mv[:, 0:1], mv[:, 0:1], bia[:, j:j + 1])
                        nc.vector.tensor_mul(mv[:, 2:3], mv[:, 0:1], mv[:, 0:1])
                        # group reduce+broadcast: [mean', var, mean'^2] x blockones/16
                        gps = tpps.tile([128, 3], F32, tag="tp")
                        nc.tensor.matmul(gps, bones[:, :], mv[:, :], start=True, stop=True)
                        gs = spool.tile([128, 3], F32, tag="gs")
                        nc.vector.tensor_copy(gs, gps[:, 0:3])
                        # var_g = Ev + Em2 - mu^2 ; scale = gamma/sqrt(var+eps)
                        t1 = spool.tile([128, 1], F32, tag="t1")
                        nc.vector.tensor_mul(t1, gs[:, 0:1], gs[:, 0:1])
                        t2 = spool.tile([128, 1], F32, tag="t2")
                        nc.vector.tensor_add(t2, gs[:, 1:2], gs[:, 2:3])
                        nc.vector.tensor_sub(t2, t2, t1)
                        nc.scalar.activation(t2, t2, AF.Sqrt, bias=eps_t[:, 0:1])
                        nc.vector.reciprocal(t2, t2)
                        scl = spool.tile([128, 1], F32, tag="scl")
                        nc.vector.tensor_mul(scl, t2, gam[:, j:j + 1])
                        # bias_eff = (bias - mu_g)*scale + beta
                        bef = spool.tile([128, 1], F32, tag="bef")
                        nc.vector.tensor_sub(bef, bia[:, j:j + 1], gs[:, 0:1])
                        nc.vector.tensor_mul(bef, bef, scl)
                        nc.vector.tensor_add(bef, bef, bet[:, j:j + 1])
                        # apply + relu, PSUM -> SBUF (rounded to fp32r)
                        yt = ypool.tile([128, T], R32, tag=f"y{j}")
                        for half in range(2):
                            nc.scalar.activation(yt[:, half * 512:half * 512 + 512],
                                                 halves[half], AF.Relu,
                                                 bias=bef[:, 0:1], scale=scl[:, 0:1])
                        y_tiles.append(yt)

                    # ---- transpose y -> xT tiles [t][128, 640] ----
                    xt_t = []
                    for t in range(NT):
                        xtt = xtpool.tile([128, CH], R32, tag=f"xt{t}")
                        tp = tpps.tile([128, 512], F32, tag="tp")
                        for j in range(4):
                            nc.tensor.transpose(
                                tp[:, j * 128:(j + 1) * 128],
                                y_tiles[j][:, t * 128:(t + 1) * 128].bitcast(F32),
                                ident[:, :])
                        nc.vector.tensor_copy(xtt[:, 0:512], tp[:, :])
                        tp2 = tpps.tile([128, 512], F32, tag="tp")
                        nc.tensor.transpose(
                            tp2[:, 0:128],
                            y_tiles[4][:, t * 128:(t + 1) * 128].bitcast(F32),
                            ident[:, :])
                        nc.vector.tensor_copy(xtt[:, 512:640], tp2[:, 0:128])
                        xt_t.append(xtt)

                    # ---- resample ----
                    if l < 2:
                        for j in range(5):
                            for hp in range(2):
                                ps = rsps.tile([128, 512], F32, tag="rs",
                                               name=f"rs{b}_{j}_{hp}")
                                for ci in range(2):
                                    c = hp * 2 + ci
                                    kt, lo = int(KT[l, c]), int(LO[l, c])
                                    for q in range(kt):
                                        nc.tensor.matmul(
                                            ps[:, ci * CHUNK:(ci + 1) * CHUNK],
                                            xt_t[lo + q][:, j * 128:(j + 1) * 128],
                                            a_sb[c][:, q * CHUNK:(q + 1) * CHUNK],
                                            start=(q == 0), stop=(q == kt - 1))
                                if CFG["rs_evict"] == "act":
                                    nc.scalar.copy(
                                        xt_x[(b, j)][:, 2 + hp * 512:2 + hp * 512 + 512],
                                        ps[:, 0:512])
                                else:
                                    nc.vector.tensor_copy(
                                        xt_x[(b, j)][:, 2 + hp * 512:2 + hp * 512 + 512],
                                        ps[:, 0:512])
                    else:
                        for t in range(NT):
                            kt, lo = int(KT2[t]), int(LO2[t])
                            psm = rsps.tile([128, 512], F32, tag="rs",
                                            name=f"rsm{b}_{t}")
                            for q in range(kt):
                                nc.tensor.matmul(
                                    psm,
                                    a_sb[t][:, q * 128:(q + 1) * 128],
                                    xt_t[lo + q][:, 0:512],
                                    start=(q == 0), stop=(q == kt - 1))
                            psf = tpps.tile([128, 512], F32, tag="tp",
                                            name=f"rsf{b}_{t}")
                            for q in range(kt):
                                nc.tensor.matmul(
                                    psf[:, 0:128],
                                    a_sb[t][:, q * 128:(q + 1) * 128],
                                    xt_t[lo + q][:, 512:640],
                                    start=(q == 0), stop=(q == kt - 1))
                            ost = opool.tile([128, CH], F32, tag="ost")
                            nc.scalar.copy(ost[:, 0:512], psm)
                            nc.vector.tensor_copy(ost[:, 512:640], psf[:, 0:128])
                            nc.gpsimd.dma_start(mel_out[b, t * 128:(t + 1) * 128, :],
                                                ost[:, 0:512])
                            nc.gpsimd.dma_start(f0_out[b, t * 128:(t + 1) * 128, :],
                                                ost[:, 512:640])
    nc.finalize()
    return nc


_CACHE = {}


def _get_program_and_inputs(inputs):
    per_core, tables = _prep_host(inputs)
    key = (tuple(tables["LO"].ravel()), tuple(tables["KT"].ravel()),
           tuple(tables["LO2"].ravel()), tuple(tables["KT2"].ravel()))
    if key not in _CACHE:
        _CACHE[key] = build_program(tables)
    return _CACHE[key], per_core


def kernel(**inputs):
    nc, per_core = _get_program_and_inputs(inputs)
    res = run_bass_kernel_spmd(nc, per_core, core_ids=list(range(NCORES)))
    mel = np.concatenate([r["mel_out"] for r in res.results], axis=0)
    f0 = np.concatenate([r["f0_out"] for r in res.results], axis=0)
    return mel, f0


def run_traced(inputs, **kw):
    """test.py helper: returns (results_object, per_core) for profiling."""
    nc, per_core = _get_program_and_inputs(inputs)
    return run_bass_kernel_spmd(nc, per_core, core_ids=list(range(NCORES)), **kw), per_core


# revision 17
# speedup vs baseline: 1.0792x; 1.0792x over previous
"""Trainium2 Bass kernel for nn_ConvNorm4ContPit: 3x (conv1d -> GroupNorm -> ReLU
-> random_resample) over two streams (cont 512ch, pit 128ch), B=32, T=1024.

Sharding: pure data parallelism, 4 examples per core across 8 cores.
Activations are channel-major [C, T] in SBUF. Conv = weight-stationary fp32r
matmuls accumulating over (ci, k) in PSUM. GroupNorm stats via bn_stats on the
conv PSUM + a block-diagonal ones matmul (partition-group reduce+broadcast),
applied fused with ReLU on ACT (per-partition scale/bias). random_resample is
an exact linear op: out = x @ A with A built on the host (<=2 nnz/column from
the gather+lerp), executed as banded fp32r matmuls on PE after transposing x
tiles (exact fp32 PE transposes).
"""
import numpy as np

import concourse.bacc as bacc_mod
import concourse.bass as bass
import concourse.mybir as mybir
import concourse.tile as tile
from concourse.bass_utils import run_bass_kernel_spmd

F32 = mybir.dt.float32
R32 = mybir.dt.float32r

B, T = 32, 1024
CONT_DIM, CONT_H = 80, 512
PIT_DIM, PIT_H = 4, 128
LAYERS = 3
MAX_PAD = 1024
MIN_SEG, MAX_SEG = 19, 32
NUM_SEG = MAX_PAD // MIN_SEG + 1   # 54
SEG_L = MAX_SEG * 2                # 64
EPS = 1e-5
NCORES = 8
BPC = B // NCORES                  # 4 examples per core
CH = CONT_H + PIT_H                # 640
NT = T // 128                      # 8 time tiles
TPAD = T + 4                       # 'same' pad 2 each side
CHUNK = 256                        # resample t_out chunk (l0/l1); >=256 for fp32r speed
NCHUNK = T // CHUNK                # 4

# tunables (overridden by experiments)
CFG = dict(
    cv_shape=512, cv_bufs=5,       # conv psum tile free width / bufs
    rs_bufs=1,                     # resample psum bufs ([128,512] each)
    tp_bufs=2,                     # transpose psum bufs ([128,512] each)
    rs_evict="act",                # engine for resample evict: act|vector
    conv_pair=False,               # interleave conv halves per weight
    tiny_engine="vector",          # engine for small stats TT ops
)


# ----------------------------------------------------------------------------
# Host-side: exact replication of the reference resample indexing -> A matrices
# ----------------------------------------------------------------------------
def _resample_matrix(scales_row, len_row):
    """A [T_in=1024, T_out=1024] f32 with A[gi,d]=1-lam, A[gi+1,d]=lam per valid
    candidate; bit-exact f32 replication of reference.random_resample."""
    idx = np.arange(SEG_L, dtype=np.float32)
    sc = scales_row.astype(np.float32).reshape(NUM_SEG, 1)
    idx_scaled = idx[None, :] / sc                       # [S, L] f32 div
    idx_fl = np.floor(idx_scaled)
    lam = (idx_scaled - idx_fl).reshape(-1)
    ln = len_row.astype(np.int64).reshape(NUM_SEG, 1)
    mask1 = idx_fl < (ln - 1).astype(np.float32)
    offset = np.cumsum(len_row.astype(np.int64))
    offset = np.concatenate([[0], offset[:-1]]).reshape(NUM_SEG, 1)
    idx_org = idx_fl.astype(np.int64) + offset
    mask2 = idx_org < (MAX_PAD - 1)
    mask = (mask1 & mask2).reshape(-1)
    mi = mask.astype(np.int64)
    dest = np.cumsum(mi) - mi
    valid = mask & (dest < MAX_PAD)
    gi = np.clip(idx_org.reshape(-1), 0, T - 2)
    A = np.zeros((T, T), dtype=np.float32)
    gv = gi[valid]
    dv = dest[valid]
    lv = lam[valid]
    A[gv, dv] = np.float32(1.0) - lv
    A[gv + 1, dv] = lv
    return A


def _band(A_all, c0, c1):
    """Union tile band [lo, hi) over a list of A matrices for out cols [c0,c1)."""
    lo_t, hi_t = NT, 0
    for A in A_all:
        rows = np.flatnonzero(np.abs(A[:, c0:c1]).sum(axis=1))
        if rows.size == 0:
            continue
        lo_t = min(lo_t, rows[0] // 128)
        hi_t = max(hi_t, rows[-1] // 128 + 1)
    if lo_t >= hi_t:
        lo_t, hi_t = 0, 1
    return int(lo_t), int(hi_t)


def _blockones():
    M = np.zeros((128, 128), dtype=np.float32)
    for g in range(8):
        M[g * 16:(g + 1) * 16, g * 16:(g + 1) * 16] = np.float32(1.0 / 16.0)
    return M


def _prep_host(inputs):
    """Build all per-core input arrays + static band tables."""
    g = {k: np.asarray(v) for k, v in inputs.items()}
    mel = g["mel"].astype(np.float32)        # [B, T, 80]
    f0 = g["f0"].astype(np.float32)          # [B, T, 4]

    # conv weights -> lhsT layout [ci, k, co]
    wc0 = np.ascontiguousarray(g["cont_w0"].transpose(1, 2, 0)).astype(np.float32)  # [80,5,512]
    wp0 = np.ascontiguousarray(g["pit_w0"].transpose(1, 2, 0)).astype(np.float32)   # [4,5,128]
    # layers 1,2: [l, ci_tile, 128, 5, co]
    wc12 = np.ascontiguousarray(
        g["cont_w"].transpose(0, 2, 3, 1).reshape(2, 4, 128, 5, CONT_H)).astype(np.float32)
    wp12 = np.ascontiguousarray(
        g["pit_w"].transpose(0, 2, 3, 1).reshape(2, 1, 128, 5, PIT_H)).astype(np.float32)

    # per-layer packs [L, 128, 5]: ctiles 0..3 = cont co slices, 4 = pit
    def pack(cont, pit):  # cont [L,512], pit [L,128] -> [L,128,5]
        out = np.empty((LAYERS, 128, 5), dtype=np.float32)
        for l in range(LAYERS):
            for j in range(4):
                out[l, :, j] = cont[l, j * 128:(j + 1) * 128]
            out[l, :, 4] = pit[l]
        return out
    bias_pack = pack(np.asarray(g["cont_b"], np.float32), np.asarray(g["pit_b"], np.float32))
    gamma_pack = pack(np.asarray(g["cont_gamma"], np.float32), np.asarray(g["pit_gamma"], np.float32))
    beta_pack = pack(np.asarray(g["cont_beta"], np.float32), np.asarray(g["pit_beta"], np.float32))

    scales = np.asarray(g["scales"], np.float32).reshape(LAYERS, B, NUM_SEG)
    len_seg = np.asarray(g["len_seg"]).reshape(LAYERS, B, NUM_SEG)

    A = [[_resample_matrix(scales[l, b], len_seg[l, b]) for b in range(B)]
         for l in range(LAYERS)]

    # static band tables (global over all B -> identical program on all cores)
    LO = np.zeros((2, NCHUNK), np.int64)
    KT = np.zeros((2, NCHUNK), np.int64)
    for l in range(2):
        for c in range(NCHUNK):
            lo, hi = _band(A[l], c * CHUNK, (c + 1) * CHUNK)
            LO[l, c], KT[l, c] = lo, hi - lo
    LO2 = np.zeros(NT, np.int64)
    KT2 = np.zeros(NT, np.int64)
    for t in range(NT):
        lo, hi = _band(A[2], t * 128, (t + 1) * 128)
        LO2[t], KT2[t] = lo, hi - lo

    # pack banded A blocks per core
    per_core = []
    for core in range(NCORES):
        bs = range(core * BPC, (core + 1) * BPC)
        m = {}
        # mel/f0 channel-major, time-padded by 2 on each side
        mel_cm = np.zeros((BPC, CONT_DIM, TPAD), np.float32)
        f0_cm = np.zeros((BPC, PIT_DIM, TPAD), np.float32)
        for i, b in enumerate(bs):
            mel_cm[i, :, 2:2 + T] = mel[b].T
            f0_cm[i, :, 2:2 + T] = f0[b].T
        m["mel"] = mel_cm
        m["f0"] = f0_cm
        m["wc0"] = wc0
        m["wp0"] = wp0
        m["wc12"] = wc12
        m["wp12"] = wp12
        m["bias_pack"] = bias_pack
        m["gamma_pack"] = gamma_pack
        m["beta_pack"] = beta_pack
        m["blockones"] = _blockones()
        m["zpad"] = np.zeros((128, 2), np.float32)
        m["ident"] = np.eye(128, dtype=np.float32)
        for l in range(2):
            for c in range(NCHUNK):
                kt, lo = int(KT[l, c]), int(LO[l, c])
                blk = np.zeros((BPC, kt, 128, CHUNK), np.float32)
                for i, b in enumerate(bs):
                    blk[i] = A[l][b][lo * 128:(lo + kt) * 128,
                                     c * CHUNK:(c + 1) * CHUNK].reshape(kt, 128, CHUNK)
                m[f"ra{l}c{c}"] = blk
        for t in range(NT):
            kt, lo = int(KT2[t]), int(LO2[t])
            blk = np.zeros((BPC, kt, 128, 128), np.float32)
            for i, b in enumerate(bs):
                blk[i] = A[2][b][lo * 128:(lo + kt) * 128,
                                 t * 128:(t + 1) * 128].reshape(kt, 128, 128)
            m[f"ra2t{t}"] = blk
        per_core.append(m)

    tables = dict(LO=LO, KT=KT, LO2=LO2, KT2=KT2)
    return per_core, tables


# ----------------------------------------------------------------------------
# Device program
# ----------------------------------------------------------------------------
def build_program(tables):
    LO, KT, LO2, KT2 = tables["LO"], tables["KT"], tables["LO2"], tables["KT2"]
    KTMAXW = max(int(KT.max()) * CHUNK, int(KT2.max()) * 128)  # A tile free width

    nc = bacc_mod.Bacc("TRN2", target_bir_lowering=False)
    AF = mybir.ActivationFunctionType

    mel_in = nc.dram_tensor("mel", [BPC, CONT_DIM, TPAD], R32, kind="ExternalInput")
    f0_in = nc.dram_tensor("f0", [BPC, PIT_DIM, TPAD], R32, kind="ExternalInput")
    wc0_in = nc.dram_tensor("wc0", [CONT_DIM, 5, CONT_H], R32, kind="ExternalInput")
    wp0_in = nc.dram_tensor("wp0", [PIT_DIM, 5, PIT_H], R32, kind="ExternalInput")
    wc12_in = nc.dram_tensor("wc12", [2, 4, 128, 5, CONT_H], R32, kind="ExternalInput")
    wp12_in = nc.dram_tensor("wp12", [2, 1, 128, 5, PIT_H], R32, kind="ExternalInput")
    bias_in = nc.dram_tensor("bias_pack", [LAYERS, 128, 5], F32, kind="ExternalInput")
    gamma_in = nc.dram_tensor("gamma_pack", [LAYERS, 128, 5], F32, kind="ExternalInput")
    beta_in = nc.dram_tensor("beta_pack", [LAYERS, 128, 5], F32, kind="ExternalInput")
    bones_in = nc.dram_tensor("blockones", [128, 128], F32, kind="ExternalInput")
    ident_in = nc.dram_tensor("ident", [128, 128], F32, kind="ExternalInput")
    ra = {}
    for l in range(2):
        for c in range(NCHUNK):
            ra[(l, c)] = nc.dram_tensor(f"ra{l}c{c}", [BPC, int(KT[l, c]), 128, CHUNK],
                                        R32, kind="ExternalInput")
    ra2 = {}
    for t in range(NT):
        ra2[t] = nc.dram_tensor(f"ra2t{t}", [BPC, int(KT2[t]), 128, 128],
                                R32, kind="ExternalInput")

    zpad_in = nc.dram_tensor("zpad", [128, 2], R32, kind="ExternalInput")
    mel_out = nc.dram_tensor("mel_out", [BPC, T, CONT_H], F32, kind="ExternalOutput")
    f0_out = nc.dram_tensor("f0_out", [BPC, T, PIT_H], F32, kind="ExternalOutput")

    with tile.TileContext(nc) as tc:
        with (
            tc.tile_pool(name="wpool", bufs=1) as wpool,
            tc.tile_pool(name="xpool", bufs=1) as xpool,
            tc.tile_pool(name="ypool", bufs=1) as ypool,
            tc.tile_pool(name="xtpool", bufs=1) as xtpool,
            tc.tile_pool(name="apool", bufs=2) as apool,
            tc.tile_pool(name="spool", bufs=2) as spool,
            tc.tile_pool(name="cpool", bufs=1) as cpool,
            tc.tile_pool(name="opool", bufs=2) as opool,
            tc.tile_pool(name="cvps", bufs=CFG["cv_bufs"], space="PSUM") as cvps,
            tc.tile_pool(name="rsps", bufs=CFG["rs_bufs"], space="PSUM") as rsps,
            tc.tile_pool(name="tpps", bufs=CFG["tp_bufs"], space="PSUM") as tpps,
        ):
            # ---- constants ----
            bones = cpool.tile([128, 128], F32, tag="bones")
            nc.sync.dma_start(bones, bones_in[:, :])
            ident = cpool.tile([128, 128], F32, tag="ident")
            nc.sync.dma_start(ident, ident_in[:, :])
            ident_r = cpool.tile([128, 128], R32, tag="identr")
            nc.vector.tensor_copy(ident_r, ident[:, :])
            eps_t = cpool.tile([128, 1], F32, tag="eps")
            nc.vector.memset(eps_t, EPS)

            # ---- persistent x tiles (conv inputs), zeroed borders ----
            xt_x = {}
            for b in range(BPC):
                for j in range(5):
                    xx = xpool.tile([128, TPAD], R32, tag=f"x_{b}_{j}")
                    xt_x[(b, j)] = xx
            # layer-0 inputs land in x_{b}_0 (mel, 80 parts), x_{b}_1 (f0, 4 parts)
            for b in range(BPC):
                nc.sync.dma_start(xt_x[(b, 0)][:CONT_DIM, :], mel_in[b])
                nc.sync.dma_start(xt_x[(b, 1)][:PIT_DIM, :], f0_in[b])
            # pad-border zeroing (needed from layer-1 on; keep off critical path)
            for b in range(BPC):
                for j in range(5):
                    xx = xt_x[(b, j)]
                    nc.gpsimd.dma_start(xx[:, 0:2], zpad_in[:, :])
                    nc.gpsimd.dma_start(xx[:, T + 2:TPAD], zpad_in[:, :])

            for l in range(LAYERS):
                # ---- load weights for this layer ----
                w_cont = []
                if l == 0:
                    w0 = wpool.tile([128, 5, CONT_H], R32, tag="wc0")
                    nc.sync.dma_start(w0[:CONT_DIM], wc0_in[:, :, :])
                    w_cont.append(w0)
                    wp = wpool.tile([128, 5, PIT_H], R32, tag="wp")
                    nc.sync.dma_start(wp[:PIT_DIM], wp0_in[:, :, :])
                else:
                    for ci_t in range(4):
                        wt = wpool.tile([128, 5, CONT_H], R32, tag=f"wc{ci_t}")
                        nc.sync.dma_start(wt, wc12_in[l - 1, ci_t])
                        w_cont.append(wt)
                    wp = wpool.tile([128, 5, PIT_H], R32, tag="wp")
                    nc.sync.dma_start(wp, wp12_in[l - 1, 0])
                gam = cpool.tile([128, 5], F32, tag="gam")
                nc.sync.dma_start(gam, gamma_in[l])
                bet = cpool.tile([128, 5], F32, tag="bet")
                nc.sync.dma_start(bet, beta_in[l])
                bia = cpool.tile([128, 5], F32, tag="bia")
                nc.sync.dma_start(bia, bias_in[l])

                # conv input tiles per stream
                def conv_srcs(b):
                    if l == 0:
                        return ([(xt_x[(b, 0)], CONT_DIM)], [(xt_x[(b, 1)], PIT_DIM)])
                    return ([(xt_x[(b, j)], 128) for j in range(4)],
                            [(xt_x[(b, 4)], 128)])

                for b in range(BPC):
                    cont_src, pit_src = conv_srcs(b)

                    # prefetch A blocks for this (l, b)
                    a_sb = {}
                    if l < 2:
                        for c in range(NCHUNK):
                            kt = int(KT[l, c])
                            at = apool.tile([128, KTMAXW], R32, tag=f"a{c % 2}")
                            nc.sync.dma_start(
                                at[:, 0:kt * CHUNK].rearrange("p (k n) -> p k n", k=kt),
                                ra[(l, c)][b].rearrange("k p n -> p k n"))
                            a_sb[c] = at
                    else:
                        for t in range(NT):
                            kt = int(KT2[t])
                            at = apool.tile([128, KTMAXW], R32, tag=f"a{t % 2}")
                            nc.sync.dma_start(
                                at[:, 0:kt * 128].rearrange("p (k n) -> p k n", k=kt),
                                ra2[t][b].rearrange("k p n -> p k n"))
                            a_sb[t] = at

                    # ---- conv + GN + ReLU per ctile ----
                    y_tiles = [None] * 5
                    for j in CFG.get("j_order", [0, 1, 2, 3, 4]):
                        is_pit = (j == 4)
                        srcs = pit_src if is_pit else cont_src
                        wlist = [wp] if is_pit else w_cont
                        co0 = 0 if is_pit else j * 128
                        if CFG["cv_shape"] == 1024:
                            pst = cvps.tile([128, 1024], F32, tag="cv")
                            halves = [pst[:, 0:512], pst[:, 512:1024]]
                        else:
                            halves = [cvps.tile([128, 512], F32, tag="cv",
                                                 name=f"cvh{b}_{j}_{h}")
                                      for h in range(2)]
                        nmm = len(srcs) * 5
                        st6 = spool.tile([128, 2, 6], F32, tag="st6")
                        if CFG.get("conv_pair", True):
                            i = 0
                            for si, (xsrc, kdim) in enumerate(srcs):
                                wt = wlist[si if not is_pit else 0] if not is_pit else wp
                                for k in range(5):
                                    for half in range(2):
                                        nc.tensor.matmul(
                                            halves[half],
                                            wt[:kdim, k, co0:co0 + 128],
                                            xsrc[:kdim, half * 512 + k:half * 512 + k + 512],
                                            start=(i == 0), stop=(i == nmm - 1))
                                    i += 1
                            for half in range(2):
                                nc.vector.bn_stats(st6[:, half, :], halves[half])
                        else:
                            for half in range(2):
                                i = 0
                                for si, (xsrc, kdim) in enumerate(srcs):
                                    wt = wlist[si if not is_pit else 0] if not is_pit else wp
                                    for k in range(5):
                                        nc.tensor.matmul(
                                            halves[half],
                                            wt[:kdim, k, co0:co0 + 128],
                                            xsrc[:kdim, half * 512 + k:half * 512 + k + 512],
                                            start=(i == 0), stop=(i == nmm - 1))
                                        i += 1
                                nc.vector.bn_stats(st6[:, half, :], halves[half])
                        mv = spool.tile([128, 3], F32, tag="mv")
                        nc.vector.bn_aggr(mv[:, 0:2], st6)
                        # mean' = mean + bias ; stash mean'^2
                        nc.vector.tensor_add(mv[:, 0:1], mv[:, 0:1], bia[:, j:j + 1])
                        nc.vector.tensor_mul(mv[:, 2:3], mv[:, 0:1], mv[:, 0:1])
                        # group reduce+broadcast: [mean', var, mean'^2] x blockones/16
                        gps = tpps.tile([128, 3], F32, tag="tp")
                        nc.tensor.matmul(gps, bones[:, :], mv[:, :], start=True, stop=True)
                        gs = spool.tile([128, 3], F32, tag="gs")
                        nc.vector.tensor_copy(gs, gps[:, 0:3])
                        # var_g = Ev + Em2 - mu^2 ; scale = gamma/sqrt(var+eps)
                        t1 = spool.tile([128, 1], F32, tag="t1")
                        nc.vector.tensor_mul(t1, gs[:, 0:1], gs[:, 0:1])
                        t2 = spool.tile([128, 1], F32, tag="t2")
                        nc.vector.tensor_add(t2, gs[:, 1:2], gs[:, 2:3])
                        nc.vector.tensor_sub(t2, t2, t1)
                        nc.scalar.activation(t2, t2, AF.Sqrt, bias=eps_t[:, 0:1])
                        nc.vector.reciprocal(t2, t2)
                        scl = spool.tile([128, 1], F32, tag="scl")
                        nc.vector.tensor_mul(scl, t2, gam[:, j:j + 1])
                        # bias_eff = (bias - mu_g)*scale + beta
                        bef = spool.tile([128, 1], F32, tag="bef")
                        nc.vector.tensor_sub(bef, bia[:, j:j + 1], gs[:, 0:1])
                        nc.vector.tensor_mul(bef, bef, scl)
                        nc.vector.tensor_add(bef, bef, bet[:, j:j + 1])
                        # apply + relu, PSUM -> SBUF (rounded to fp32r)
                        yt = ypool.tile([128, T], R32, tag=f"y{j}")
                        for half in range(2):
                            nc.scalar.activation(yt[:, half * 512:half * 512 + 512],
                                                 halves[half], AF.Relu,
                                                 bias=bef[:, 0:1], scale=scl[:, 0:1])
                        y_tiles[j] = yt

                    # ---- transpose y -> xT [128, t, 640] (one tensor per b) ----
                    xtall = xtpool.tile([128, NT, CH], R32, tag="xtall",
                                        name=f"xtall{b}")
                    for t in range(NT):
                        tp = tpps.tile([128, 512], F32, tag="tp",
                                       name=f"tpa{b}_{t}")
                        for j in range(4):
                            nc.tensor.transpose(
                                tp[:, j * 128:(j + 1) * 128].bitcast(R32),
                                y_tiles[j][:, t * 128:(t + 1) * 128],
                                ident_r[:, :])
                        nc.scalar.copy(xtall[:, t, 0:512], tp[:, :])
                    for tg in range(2):
                        tp2 = tpps.tile([128, 512], F32, tag="tp",
                                        name=f"tpb{b}_{tg}")
                        for dt_ in range(4):
                            nc.tensor.transpose(
                                tp2[:, dt_ * 128:(dt_ + 1) * 128].bitcast(R32),
                                y_tiles[4][:, (tg * 4 + dt_) * 128:(tg * 4 + dt_ + 1) * 128],
                                ident_r[:, :])
                        nc.vector.tensor_copy(
                            xtall[:, tg * 4:(tg + 1) * 4, 512:640],
                            tp2[:, :].rearrange("p (t n) -> p t n", t=4))
                    xt_t = [xtall[:, t, :] for t in range(NT)]

                    # ---- resample ----
                    if l < 2:
                        for j in range(5):
                            for hp in range(2):
                                ps = rsps.tile([128, 512], F32, tag="rs",
                                               name=f"rs{b}_{j}_{hp}")
                                for ci in range(2):
                                    c = hp * 2 + ci
                                    kt, lo = int(KT[l, c]), int(LO[l, c])
                                    for q in range(kt):
                                        nc.tensor.matmul(
                                            ps[:, ci * CHUNK:(ci + 1) * CHUNK],
                                            xt_t[lo + q][:, j * 128:(j + 1) * 128],
                                            a_sb[c][:, q * CHUNK:(q + 1) * CHUNK],
                                            start=(q == 0), stop=(q == kt - 1))
                                if CFG["rs_evict"] == "act":
                                    nc.scalar.copy(
                                        xt_x[(b, j)][:, 2 + hp * 512:2 + hp * 512 + 512],
                                        ps[:, 0:512])
                                else:
                                    nc.vector.tensor_copy(
                                        xt_x[(b, j)][:, 2 + hp * 512:2 + hp * 512 + 512],
                                        ps[:, 0:512])
                    else:
                        for t in range(NT):
                            kt, lo = int(KT2[t]), int(LO2[t])
                            psm = rsps.tile([128, 512], F32, tag="rs",
                                            name=f"rsm{b}_{t}")
                            for q in range(kt):
                                nc.tensor.matmul(
                                    psm,
                                    a_sb[t][:, q * 128:(q + 1) * 128],
                                    xt_t[lo + q][:, 0:512],
                                    start=(q == 0), stop=(q == kt - 1))
                            psf = tpps.tile([128, 512], F32, tag="tp",
                                            name=f"rsf{b}_{t}")
                            for q in range(kt):
                                nc.tensor.matmul(
                                    psf[:, 0:128],
                                    a_sb[t][:, q * 128:(q + 1) * 128],
                                    xt_t[lo + q][:, 512:640],
                                    start=(q == 0), stop=(q == kt - 1))
                            ost = opool.tile([128, CH], F32, tag="ost")
                            nc.scalar.copy(ost[:, 0:512], psm)
                            nc.vector.tensor_copy(ost[:, 512:640], psf[:, 0:128])
                            nc.gpsimd.dma_start(mel_out[b, t * 128:(t + 1) * 128, :],
                                                ost[:, 0:512])
                            nc.gpsimd.dma_start(f0_out[b, t * 128:(t + 1) * 128, :],
                                                ost[:, 512:640])
    nc.finalize()
    return nc


_CACHE = {}


def _get_program_and_inputs(inputs):
    per_core, tables = _prep_host(inputs)
    key = (tuple(tables["LO"].ravel()), tuple(tables["KT"].ravel()),
           tuple(tables["LO2"].ravel()), tuple(tables["KT2"].ravel()))
    if key not in _CACHE:
        _CACHE[key] = build_program(tables)
    return _CACHE[key], per_core


def kernel(**inputs):
    nc, per_core = _get_program_and_inputs(inputs)
    res = run_bass_kernel_spmd(nc, per_core, core_ids=list(range(NCORES)))
    mel = np.concatenate([r["mel_out"] for r in res.results], axis=0)
    f0 = np.concatenate([r["f0_out"] for r in res.results], axis=0)
    return mel, f0


def run_traced(inputs, **kw):
    """test.py helper: returns (results_object, per_core) for profiling."""
    nc, per_core = _get_program_and_inputs(inputs)
    return run_bass_kernel_spmd(nc, per_core, core_ids=list(range(NCORES)), **kw), per_core


# revision 18
# speedup vs baseline: 11343.0173x; 10510.5259x over previous
"""Trainium2 Bass kernel for nn_ConvNorm4ContPit: 3x (conv1d -> GroupNorm -> ReLU
-> random_resample) over two streams (cont 512ch, pit 128ch), B=32, T=1024.

Sharding: pure data parallelism, 4 examples per core across 8 cores.
Activations are channel-major [C, T] in SBUF. Conv = weight-stationary fp32r
matmuls accumulating over (ci, k) in PSUM. GroupNorm stats via bn_stats on the
conv PSUM + a block-diagonal ones matmul (partition-group reduce+broadcast),
applied fused with ReLU on ACT (per-partition scale/bias). random_resample is
an exact linear op: out = x @ A with A built on the host (<=2 nnz/column from
the gather+lerp), executed as banded fp32r matmuls on PE after transposing x
tiles (exact fp32 PE transposes).
"""
import numpy as np

import concourse.bacc as bacc_mod
import concourse.mybir as mybir
import concourse.tile as tile
from concourse.bass_utils import run_bass_kernel_spmd

F32 = mybir.dt.float32
R32 = mybir.dt.float32r

B, T = 32, 1024
CONT_DIM, CONT_H = 80, 512
PIT_DIM, PIT_H = 4, 128
LAYERS = 3
MAX_PAD = 1024
MIN_SEG, MAX_SEG = 19, 32
NUM_SEG = MAX_PAD // MIN_SEG + 1   # 54
SEG_L = MAX_SEG * 2                # 64
EPS = 1e-5
NCORES = 8
BPC = B // NCORES                  # 4 examples per core
CH = CONT_H + PIT_H                # 640
NT = T // 128                      # 8 time tiles
TPAD = T + 4                       # 'same' pad 2 each side
CHUNK = 256                        # resample t_out chunk (l0/l1); >=256 for fp32r speed
NCHUNK = T // CHUNK                # 4

# tunables (overridden by experiments)
CFG = dict(
    cv_shape=512, cv_bufs=5,       # conv psum tile free width / bufs
    rs_bufs=1,                     # resample psum bufs ([128,512] each)
    tp_bufs=2,                     # transpose psum bufs ([128,512] each)
    rs_evict="act",                # engine for resample evict: act|vector
    conv_pair=False,               # interleave conv halves per weight
    tiny_engine="vector",          # engine for small stats TT ops
)


# ----------------------------------------------------------------------------
# Host-side: exact replication of the reference resample indexing -> A matrices
# ----------------------------------------------------------------------------
def _resample_matrix(scales_row, len_row):
    """A [T_in=1024, T_out=1024] f32 with A[gi,d]=1-lam, A[gi+1,d]=lam per valid
    candidate; bit-exact f32 replication of reference.random_resample."""
    idx = np.arange(SEG_L, dtype=np.float32)
    sc = scales_row.astype(np.float32).reshape(NUM_SEG, 1)
    idx_scaled = idx[None, :] / sc                       # [S, L] f32 div
    idx_fl = np.floor(idx_scaled)
    lam = (idx_scaled - idx_fl).reshape(-1)
    ln = len_row.astype(np.int64).reshape(NUM_SEG, 1)
    mask1 = idx_fl < (ln - 1).astype(np.float32)
    offset = np.cumsum(len_row.astype(np.int64))
    offset = np.concatenate([[0], offset[:-1]]).reshape(NUM_SEG, 1)
    idx_org = idx_fl.astype(np.int64) + offset
    mask2 = idx_org < (MAX_PAD - 1)
    mask = (mask1 & mask2).reshape(-1)
    mi = mask.astype(np.int64)
    dest = np.cumsum(mi) - mi
    valid = mask & (dest < MAX_PAD)
    gi = np.clip(idx_org.reshape(-1), 0, T - 2)
    A = np.zeros((T, T), dtype=np.float32)
    gv = gi[valid]
    dv = dest[valid]
    lv = lam[valid]
    A[gv, dv] = np.float32(1.0) - lv
    A[gv + 1, dv] = lv
    return A


def _band(A_all, c0, c1):
    """Union tile band [lo, hi) over a list of A matrices for out cols [c0,c1)."""
    lo_t, hi_t = NT, 0
    for A in A_all:
        rows = np.flatnonzero(np.abs(A[:, c0:c1]).sum(axis=1))
        if rows.size == 0:
            continue
        lo_t = min(lo_t, rows[0] // 128)
        hi_t = max(hi_t, rows[-1] // 128 + 1)
    if lo_t >= hi_t:
        lo_t, hi_t = 0, 1
    return int(lo_t), int(hi_t)


def _blockones():
    M = np.zeros((128, 128), dtype=np.float32)
    for g in range(8):
        M[g * 16:(g + 1) * 16, g * 16:(g + 1) * 16] = np.float32(1.0 / 16.0)
    return M


def _prep_host(inputs):
    """Build all per-core input arrays + static band tables."""
    g = {k: np.asarray(v) for k, v in inputs.items()}
    mel = g["mel"].astype(np.float32)        # [B, T, 80]
    f0 = g["f0"].astype(np.float32)          # [B, T, 4]

    # conv weights -> lhsT layout [ci, k, co]
    wc0 = np.ascontiguousarray(g["cont_w0"].transpose(1, 2, 0)).astype(np.float32)  # [80,5,512]
    wp0 = np.ascontiguousarray(g["pit_w0"].transpose(1, 2, 0)).astype(np.float32)   # [4,5,128]
    # layers 1,2: [l, ci_tile, 128, 5, co]
    wc12 = np.ascontiguousarray(
        g["cont_w"].transpose(0, 2, 3, 1).reshape(2, 4, 128, 5, CONT_H)).astype(np.float32)
    wp12 = np.ascontiguousarray(
        g["pit_w"].transpose(0, 2, 3, 1).reshape(2, 1, 128, 5, PIT_H)).astype(np.float32)

    # per-layer packs [L, 128, 5]: ctiles 0..3 = cont co slices, 4 = pit
    def pack(cont, pit):  # cont [L,512], pit [L,128] -> [L,128,5]
        out = np.empty((LAYERS, 128, 5), dtype=np.float32)
        for l in range(LAYERS):
            for j in range(4):
                out[l, :, j] = cont[l, j * 128:(j + 1) * 128]
            out[l, :, 4] = pit[l]
        return out
    bias_pack = pack(np.asarray(g["cont_b"], np.float32), np.asarray(g["pit_b"], np.float32))
    gamma_pack = pack(np.asarray(g["cont_gamma"], np.float32), np.asarray(g["pit_gamma"], np.float32))
    beta_pack = pack(np.asarray(g["cont_beta"], np.float32), np.asarray(g["pit_beta"], np.float32))

    scales = np.asarray(g["scales"], np.float32).reshape(LAYERS, B, NUM_SEG)
    len_seg = np.asarray(g["len_seg"]).reshape(LAYERS, B, NUM_SEG)

    A = [[_resample_matrix(scales[l, b], len_seg[l, b]) for b in range(B)]
         for l in range(LAYERS)]

    # static band tables (global over all B -> identical program on all cores)
    LO = np.zeros((2, NCHUNK), np.int64)
    KT = np.zeros((2, NCHUNK), np.int64)
    for l in range(2):
        for c in range(NCHUNK):
            lo, hi = _band(A[l], c * CHUNK, (c + 1) * CHUNK)
            LO[l, c], KT[l, c] = lo, hi - lo
    LO2 = np.zeros(NT, np.int64)
    KT2 = np.zeros(NT, np.int64)
    for t in range(NT):
        lo, hi = _band(A[2], t * 128, (t + 1) * 128)
        LO2[t], KT2[t] = lo, hi - lo

    # pack banded A blocks per core
    per_core = []
    for core in range(NCORES):
        bs = range(core * BPC, (core + 1) * BPC)
        m = {}
        # mel/f0 channel-major, time-padded by 2 on each side
        mel_cm = np.zeros((BPC, CONT_DIM, TPAD), np.float32)
        f0_cm = np.zeros((BPC, PIT_DIM, TPAD), np.float32)
        for i, b in enumerate(bs):
            mel_cm[i, :, 2:2 + T] = mel[b].T
            f0_cm[i, :, 2:2 + T] = f0[b].T
        m["mel"] = mel_cm
        m["f0"] = f0_cm
        m["wc0"] = wc0
        m["wp0"] = wp0
        m["wc12"] = wc12
        m["wp12"] = wp12
        m["bias_pack"] = bias_pack
        m["gamma_pack"] = gamma_pack
        m["beta_pack"] = beta_pack
        m["blockones"] = _blockones()
        m["zpad"] = np.zeros((128, 2), np.float32)
        m["ident"] = np.eye(128, dtype=np.float32)
        for l in range(2):
            for c in range(NCHUNK):
                kt, lo = int(KT[l, c]), int(LO[l, c])
                blk = np.zeros((BPC, kt, 128, CHUNK), np.float32)
                for i, b in enumerate(bs):
                    blk[i] = A[l][b][lo * 128:(lo + kt) * 128,
                                     c * CHUNK:(c + 1) * CHUNK].reshape(kt, 128, CHUNK)
                m[f"ra{l}c{c}"] = blk
        for t in range(NT):
            kt, lo = int(KT2[t]), int(LO2[t])
            blk = np.zeros((BPC, kt, 128, 128), np.float32)
            for i, b in enumerate(bs):
                blk[i] = A[2][b][lo * 128:(lo + kt) * 128,
                                 t * 128:(t + 1) * 128].reshape(kt, 128, 128)
            m[f"ra2t{t}"] = blk
        per_core.append(m)

    tables = dict(LO=LO, KT=KT, LO2=LO2, KT2=KT2)
    return per_core, tables


# ----------------------------------------------------------------------------
# Device program
# ----------------------------------------------------------------------------
def build_program(tables):
    LO, KT, LO2, KT2 = tables["LO"], tables["KT"], tables["LO2"], tables["KT2"]
    KTMAXW = max(int(KT.max()) * CHUNK, int(KT2.max()) * 128)  # A tile free width

    nc = bacc_mod.Bacc("TRN2", target_bir_lowering=False)
    AF = mybir.ActivationFunctionType

    mel_in = nc.dram_tensor("mel", [BPC, CONT_DIM, TPAD], R32, kind="ExternalInput")
    f0_in = nc.dram_tensor("f0", [BPC, PIT_DIM, TPAD], R32, kind="ExternalInput")
    wc0_in = nc.dram_tensor("wc0", [CONT_DIM, 5, CONT_H], R32, kind="ExternalInput")
    wp0_in = nc.dram_tensor("wp0", [PIT_DIM, 5, PIT_H], R32, kind="ExternalInput")
    wc12_in = nc.dram_tensor("wc12", [2, 4, 128, 5, CONT_H], R32, kind="ExternalInput")
    wp12_in = nc.dram_tensor("wp12", [2, 1, 128, 5, PIT_H], R32, kind="ExternalInput")
    bias_in = nc.dram_tensor("bias_pack", [LAYERS, 128, 5], F32, kind="ExternalInput")
    gamma_in = nc.dram_tensor("gamma_pack", [LAYERS, 128, 5], F32, kind="ExternalInput")
    beta_in = nc.dram_tensor("beta_pack", [LAYERS, 128, 5], F32, kind="ExternalInput")
    bones_in = nc.dram_tensor("blockones", [128, 128], F32, kind="ExternalInput")
    ident_in = nc.dram_tensor("ident", [128, 128], F32, kind="ExternalInput")
    ra = {}
    for l in range(2):
        for c in range(NCHUNK):
            ra[(l, c)] = nc.dram_tensor(f"ra{l}c{c}", [BPC, int(KT[l, c]), 128, CHUNK],
                                        R32, kind="ExternalInput")
    ra2 = {}
    for t in range(NT):
        ra2[t] = nc.dram_tensor(f"ra2t{t}", [BPC, int(KT2[t]), 128, 128],
                                R32, kind="ExternalInput")

    zpad_in = nc.dram_tensor("zpad", [128, 2], R32, kind="ExternalInput")
    mel_out = nc.dram_tensor("mel_out", [BPC, T, CONT_H], F32, kind="ExternalOutput")
    f0_out = nc.dram_tensor("f0_out", [BPC, T, PIT_H], F32, kind="ExternalOutput")

    with tile.TileContext(nc) as tc:
        with (
            tc.tile_pool(name="wpool", bufs=1) as wpool,
            tc.tile_pool(name="xpool", bufs=1) as xpool,
            tc.tile_pool(name="ypool", bufs=1) as ypool,
            tc.tile_pool(name="xtpool", bufs=1) as xtpool,
            tc.tile_pool(name="apool", bufs=2) as apool,
            tc.tile_pool(name="spool", bufs=2) as spool,
            tc.tile_pool(name="cpool", bufs=1) as cpool,
            tc.tile_pool(name="opool", bufs=2) as opool,
            tc.tile_pool(name="cvps", bufs=CFG["cv_bufs"], space="PSUM") as cvps,
            tc.tile_pool(name="rsps", bufs=CFG["rs_bufs"], space="PSUM") as rsps,
            tc.tile_pool(name="tpps", bufs=CFG["tp_bufs"], space="PSUM") as tpps,
        ):
            # ---- constants ----
            bones = cpool.tile([128, 128], F32, tag="bones")
            nc.sync.dma_start(bones, bones_in[:, :])
            ident = cpool.tile([128, 128], F32, tag="ident")
            nc.sync.dma_start(ident, ident_in[:, :])
            ident_r = cpool.tile([128, 128], R32, tag="identr")
            nc.vector.tensor_copy(ident_r, ident[:, :])
            eps_t = cpool.tile([128, 1], F32, tag="eps")
            nc.vector.memset(eps_t, EPS)

            # ---- persistent x tiles (conv inputs), zeroed borders ----
            xt_x = {}
            for b in range(BPC):
                for j in range(5):
                    xx = xpool.tile([128, TPAD], R32, tag=f"x_{b}_{j}")
                    xt_x[(b, j)] = xx
            # layer-0 inputs land in x_{b}_0 (mel, 80 parts), x_{b}_1 (f0, 4 parts)
            for b in range(BPC):
                nc.sync.dma_start(xt_x[(b, 0)][:CONT_DIM, :], mel_in[b])
                nc.sync.dma_start(xt_x[(b, 1)][:PIT_DIM, :], f0_in[b])
            # pad-border zeroing (needed from layer-1 on; keep off critical path)
            for b in range(BPC):
                for j in range(5):
                    xx = xt_x[(b, j)]
                    nc.gpsimd.dma_start(xx[:, 0:2], zpad_in[:, :])
                    nc.gpsimd.dma_start(xx[:, T + 2:TPAD], zpad_in[:, :])

            for l in range(LAYERS):
                # ---- load weights for this layer ----
                w_cont = []
                if l == 0:
                    w0 = wpool.tile([128, 5, CONT_H], R32, tag="wc0")
                    nc.sync.dma_start(w0[:CONT_DIM], wc0_in[:, :, :])
                    w_cont.append(w0)
                    wp = wpool.tile([128, 5, PIT_H], R32, tag="wp")
                    nc.sync.dma_start(wp[:PIT_DIM], wp0_in[:, :, :])
                else:
                    for ci_t in range(4):
                        wt = wpool.tile([128, 5, CONT_H], R32, tag=f"wc{ci_t}")
                        nc.sync.dma_start(wt, wc12_in[l - 1, ci_t])
                        w_cont.append(wt)
                    wp = wpool.tile([128, 5, PIT_H], R32, tag="wp")
                    nc.sync.dma_start(wp, wp12_in[l - 1, 0])
                gam = cpool.tile([128, 5], F32, tag="gam")
                nc.sync.dma_start(gam, gamma_in[l])
                bet = cpool.tile([128, 5], F32, tag="bet")
                nc.sync.dma_start(bet, beta_in[l])
                bia = cpool.tile([128, 5], F32, tag="bia")
                nc.sync.dma_start(bia, bias_in[l])

                # conv input tiles per stream
                def conv_srcs(b):
                    if l == 0:
                        return ([(xt_x[(b, 0)], CONT_DIM)], [(xt_x[(b, 1)], PIT_DIM)])
                    return ([(xt_x[(b, j)], 128) for j in range(4)],
                            [(xt_x[(b, 4)], 128)])

                for b in range(BPC):
                    cont_src, pit_src = conv_srcs(b)

                    # prefetch A blocks for this (l, b)
                    a_sb = {}
                    if l < 2:
                        for c in range(NCHUNK):
                            kt = int(KT[l, c])
                            at = apool.tile([128, KTMAXW], R32, tag=f"a{c % 2}")
                            nc.sync.dma_start(
                                at[:, 0:kt * CHUNK].rearrange("p (k n) -> p k n", k=kt),
                                ra[(l, c)][b].rearrange("k p n -> p k n"))
                            a_sb[c] = at
                    else:
                        for t in range(NT):
                            kt = int(KT2[t])
                            at = apool.tile([128, KTMAXW], R32, tag=f"a{t % 2}")
                            nc.sync.dma_start(
                                at[:, 0:kt * 128].rearrange("p (k n) -> p k n", k=kt),
                                ra2[t][b].rearrange("k p n -> p k n"))
                            a_sb[t] = at

                    # ---- conv + GN + ReLU per ctile ----
                    y_tiles = [None] * 5
                    for j in CFG.get("j_order", [0, 1, 2, 3, 4]):
                        is_pit = (j == 4)
                        srcs = pit_src if is_pit else cont_src
                        wlist = [wp] if is_pit else w_cont
                        co0 = 0 if is_pit else j * 128
                        if CFG["cv_shape"] == 1024:
                            pst = cvps.tile([128, 1024], F32, tag="cv")
                            halves = [pst[:, 0:512], pst[:, 512:1024]]
                        else:
                            halves = [cvps.tile([128, 512], F32, tag="cv",
                                                 name=f"cvh{b}_{j}_{h}")
                                      for h in range(2)]
                        nmm = len(srcs) * 5
                        st6 = spool.tile([128, 2, 6], F32, tag="st6")
                        if CFG.get("conv_pair", True):
                            i = 0
                            for si, (xsrc, kdim) in enumerate(srcs):
                                wt = wlist[si if not is_pit else 0] if not is_pit else wp
                                for k in range(5):
                                    for half in range(2):
                                        nc.tensor.matmul(
                                            halves[half],
                                            wt[:kdim, k, co0:co0 + 128],
                                            xsrc[:kdim, half * 512 + k:half * 512 + k + 512],
                                            start=(i == 0), stop=(i == nmm - 1))
                                    i += 1
                            for half in range(2):
                                nc.vector.bn_stats(st6[:, half, :], halves[half])
                        else:
                            for half in range(2):
                                i = 0
                                for si, (xsrc, kdim) in enumerate(srcs):
                                    wt = wlist[si if not is_pit else 0] if not is_pit else wp
                                    for k in range(5):
                                        nc.tensor.matmul(
                                            halves[half],
                                            wt[:kdim, k, co0:co0 + 128],
                                            xsrc[:kdim, half * 512 + k:half * 512 + k + 512],
                                            start=(i == 0), stop=(i == nmm - 1))
                                        i += 1
                                nc.vector.bn_stats(st6[:, half, :], halves[half])
                        mv = spool.tile([128, 3], F32, tag="mv")
                        nc.vector.bn_aggr(mv[:, 0:2], st6)
                        # mean' = mean + bias ; stash mean'^2
                        nc.vector.tensor_add(mv[:, 0:1], mv[:, 0:1], bia[:, j:j + 1])
                        nc.vector.tensor_mul(mv[:, 2:3], mv[:, 0:1], mv[:, 0:1])
                        # group reduce+broadcast: [mean', var, mean'^2] x blockones/16
                        gps = tpps.tile([128, 3], F32, tag="tp")
                        nc.tensor.matmul(gps, bones[:, :], mv[:, :], start=True, stop=True)
                        gs = spool.tile([128, 3], F32, tag="gs")
                        nc.vector.tensor_copy(gs, gps[:, 0:3])
                        # var_g = Ev + Em2 - mu^2 ; scale = gamma/sqrt(var+eps)
                        t1 = spool.tile([128, 1], F32, tag="t1")
                        nc.vector.tensor_mul(t1, gs[:, 0:1], gs[:, 0:1])
                        t2 = spool.tile([128, 1], F32, tag="t2")
                        nc.vector.tensor_add(t2, gs[:, 1:2], gs[:, 2:3])
                        nc.vector.tensor_sub(t2, t2, t1)
                        nc.scalar.activation(t2, t2, AF.Sqrt, bias=eps_t[:, 0:1])
                        nc.vector.reciprocal(t2, t2)
                        scl = spool.tile([128, 1], F32, tag="scl")
                        nc.vector.tensor_mul(scl, t2, gam[:, j:j + 1])
                        # bias_eff = (bias - mu_g)*scale + beta
                        bef = spool.tile([128, 1], F32, tag="bef")
                        nc.vector.tensor_sub(bef, bia[:, j:j + 1], gs[:, 0:1])
                        nc.vector.tensor_mul(bef, bef, scl)
                        nc.vector.tensor_add(bef, bef, bet[:, j:j + 1])
                        # apply + relu, PSUM -> SBUF (rounded to fp32r)
                        yt = ypool.tile([128, T], R32, tag=f"y{j}")
                        for half in range(2):
                            nc.scalar.activation(yt[:, half * 512:half * 512 + 512],
                                                 halves[half], AF.Relu,
                                                 bias=bef[:, 0:1], scale=scl[:, 0:1])
                        y_tiles[j] = yt

                    # ---- transpose y -> xT [128, t, 640] (one tensor per b) ----
                    xtall = xtpool.tile([128, NT, CH], R32, tag="xtall",
                                        name=f"xtall{b}")
                    for t in range(NT):
                        tp = tpps.tile([128, 512], F32, tag="tp",
                                       name=f"tpa{b}_{t}")
                        for j in range(4):
                            nc.tensor.transpose(
                                tp[:, j * 128:(j + 1) * 128].bitcast(R32),
                                y_tiles[j][:, t * 128:(t + 1) * 128],
                                ident_r[:, :])
                        nc.scalar.copy(xtall[:, t, 0:512], tp[:, :])
                    for tg in range(2):
                        tp2 = tpps.tile([128, 512], F32, tag="tp",
                                        name=f"tpb{b}_{tg}")
                        for dt_ in range(4):
                            nc.tensor.transpose(
                                tp2[:, dt_ * 128:(dt_ + 1) * 128].bitcast(R32),
                                y_tiles[4][:, (tg * 4 + dt_) * 128:(tg * 4 + dt_ + 1) * 128],
                                ident_r[:, :])
                        nc.vector.tensor_copy(
                            xtall[:, tg * 4:(tg + 1) * 4, 512:640],
                            tp2[:, :].rearrange("p (t n) -> p t n", t=4))
                    xt_t = [xtall[:, t, :] for t in range(NT)]

                    # ---- resample ----
                    if l < 2:
                        for j in range(5):
                            for hp in range(2):
                                ps = rsps.tile([128, 512], F32, tag="rs",
                                               name=f"rs{b}_{j}_{hp}")
                                for ci in range(2):
                                    c = hp * 2 + ci
                                    kt, lo = int(KT[l, c]), int(LO[l, c])
                                    for q in range(kt):
                                        nc.tensor.matmul(
                                            ps[:, ci * CHUNK:(ci + 1) * CHUNK],
                                            xt_t[lo + q][:, j * 128:(j + 1) * 128],
                                            a_sb[c][:, q * CHUNK:(q + 1) * CHUNK],
                                            start=(q == 0), stop=(q == kt - 1))
                                if CFG["rs_evict"] == "act":
                                    nc.scalar.copy(
                                        xt_x[(b, j)][:, 2 + hp * 512:2 + hp * 512 + 512],
                                        ps[:, 0:512])
                                else:
                                    nc.vector.tensor_copy(
                                        xt_x[(b, j)][:, 2 + hp * 512:2 + hp * 512 + 512],
                                        ps[:, 0:512])
                    else:
                        for t in range(NT):
                            kt, lo = int(KT2[t]), int(LO2[t])
                            psm = rsps.tile([128, 512], F32, tag="rs",
                                            name=f"rsm{b}_{t}")
                            for q in range(kt):
                                nc.tensor.matmul(
                                    psm,
                                    a_sb[t][:, q * 128:(q + 1) * 128],
                                    xt_t[lo + q][:, 0:512],
                                    start=(q == 0), stop=(q == kt - 1))
                            psf = tpps.tile([128, 512], F32, tag="tp",
                                            name=f"rsf{b}_{t}")
                            for q in range(kt):
                                nc.tensor.matmul(
                                    psf[:, 0:128],
                                    a_sb[t][:, q * 128:(q + 1) * 128],
                                    xt_t[lo + q][:, 512:640],
                                    start=(q == 0), stop=(q == kt - 1))
                            ost = opool.tile([128, CH], F32, tag="ost")
                            nc.scalar.copy(ost[:, 0:512], psm)
                            nc.vector.tensor_copy(ost[:, 512:640], psf[:, 0:128])
                            nc.gpsimd.dma_start(mel_out[b, t * 128:(t + 1) * 128, :],
                                                ost[:, 0:512])
                            nc.gpsimd.dma_start(f0_out[b, t * 128:(t + 1) * 128, :],
                                                ost[:, 512:640])
    nc.finalize()
    return nc


_CACHE = {}


def _get_program_and_inputs(inputs):
    per_core, tables = _prep_host(inputs)
    key = (tuple(tables["LO"].ravel()), tuple(tables["KT"].ravel()),
           tuple(tables["LO2"].ravel()), tuple(tables["KT2"].ravel()))
    if key not in _CACHE:
        _CACHE[key] = build_program(tables)
    return _CACHE[key], per_core


def kernel(**inputs):
    nc, per_core = _get_program_and_inputs(inputs)
    res = run_bass_kernel_spmd(nc, per_core, core_ids=list(range(NCORES)))
    mel = np.concatenate([r["mel_out"] for r in res.results], axis=0)
    f0 = np.concatenate([r["f0_out"] for r in res.results], axis=0)
    return mel, f0


def run_traced(inputs, **kw):
    """test.py helper: returns (results_object, per_core) for profiling."""
    nc, per_core = _get_program_and_inputs(inputs)
    return run_bass_kernel_spmd(nc, per_core, core_ids=list(range(NCORES)), **kw), per_core


# revision 19
# speedup vs baseline: 11595.0878x; 1.0222x over previous
"""Trainium2 Bass kernel for nn_ConvNorm4ContPit: 3x (conv1d -> GroupNorm -> ReLU
-> random_resample) over two streams (cont 512ch, pit 128ch), B=32, T=1024.

Sharding: pure data parallelism, 4 examples per core across 8 cores.
Activations are channel-major [C, T] in SBUF. Conv = weight-stationary fp32r
matmuls accumulating over (ci, k) in PSUM. GroupNorm stats via bn_stats on the
conv PSUM + a block-diagonal ones matmul (partition-group reduce+broadcast),
applied fused with ReLU on ACT (per-partition scale/bias). random_resample is
an exact linear op: out = x @ A with A built on the host (<=2 nnz/column from
the gather+lerp), executed as banded fp32r matmuls on PE after transposing x
tiles (exact fp32 PE transposes).
"""
import numpy as np

import concourse.bacc as bacc_mod
import concourse.mybir as mybir
import concourse.tile as tile
from concourse.bass_utils import run_bass_kernel_spmd

F32 = mybir.dt.float32
R32 = mybir.dt.float32r

B, T = 32, 1024
CONT_DIM, CONT_H = 80, 512
PIT_DIM, PIT_H = 4, 128
LAYERS = 3
MAX_PAD = 1024
MIN_SEG, MAX_SEG = 19, 32
NUM_SEG = MAX_PAD // MIN_SEG + 1   # 54
SEG_L = MAX_SEG * 2                # 64
EPS = 1e-5
NCORES = 8
BPC = B // NCORES                  # 4 examples per core
CH = CONT_H + PIT_H                # 640
NT = T // 128                      # 8 time tiles
TPAD = T + 4                       # 'same' pad 2 each side
CHUNK = 256                        # resample t_out chunk (l0/l1); >=256 for fp32r speed
NCHUNK = T // CHUNK                # 4

# tunables (overridden by experiments)
CFG = dict(
    cv_shape=512, cv_bufs=5,       # conv psum tile free width / bufs
    rs_bufs=1,                     # resample psum bufs ([128,512] each)
    tp_bufs=2,                     # transpose psum bufs ([128,512] each)
    rs_evict="act",                # engine for resample evict: act|vector
    conv_pair=False,               # interleave conv halves per weight
    tiny_engine="vector",          # engine for small stats TT ops
)


# ----------------------------------------------------------------------------
# Host-side: exact replication of the reference resample indexing -> A matrices
# ----------------------------------------------------------------------------
def _resample_matrix(scales_row, len_row):
    """A [T_in=1024, T_out=1024] f32 with A[gi,d]=1-lam, A[gi+1,d]=lam per valid
    candidate; bit-exact f32 replication of reference.random_resample."""
    idx = np.arange(SEG_L, dtype=np.float32)
    sc = scales_row.astype(np.float32).reshape(NUM_SEG, 1)
    idx_scaled = idx[None, :] / sc                       # [S, L] f32 div
    idx_fl = np.floor(idx_scaled)
    lam = (idx_scaled - idx_fl).reshape(-1)
    ln = len_row.astype(np.int64).reshape(NUM_SEG, 1)
    mask1 = idx_fl < (ln - 1).astype(np.float32)
    offset = np.cumsum(len_row.astype(np.int64))
    offset = np.concatenate([[0], offset[:-1]]).reshape(NUM_SEG, 1)
    idx_org = idx_fl.astype(np.int64) + offset
    mask2 = idx_org < (MAX_PAD - 1)
    mask = (mask1 & mask2).reshape(-1)
    mi = mask.astype(np.int64)
    dest = np.cumsum(mi) - mi
    valid = mask & (dest < MAX_PAD)
    gi = np.clip(idx_org.reshape(-1), 0, T - 2)
    A = np.zeros((T, T), dtype=np.float32)
    gv = gi[valid]
    dv = dest[valid]
    lv = lam[valid]
    A[gv, dv] = np.float32(1.0) - lv
    A[gv + 1, dv] = lv
    return A


def _band(A_all, c0, c1):
    """Union tile band [lo, hi) over a list of A matrices for out cols [c0,c1)."""
    lo_t, hi_t = NT, 0
    for A in A_all:
        rows = np.flatnonzero(np.abs(A[:, c0:c1]).sum(axis=1))
        if rows.size == 0:
            continue
        lo_t = min(lo_t, rows[0] // 128)
        hi_t = max(hi_t, rows[-1] // 128 + 1)
    if lo_t >= hi_t:
        lo_t, hi_t = 0, 1
    return int(lo_t), int(hi_t)


def _blockones():
    M = np.zeros((128, 128), dtype=np.float32)
    for g in range(8):
        M[g * 16:(g + 1) * 16, g * 16:(g + 1) * 16] = np.float32(1.0 / 16.0)
    return M


def _prep_host(inputs):
    """Build all per-core input arrays + static band tables."""
    g = {k: np.asarray(v) for k, v in inputs.items()}
    mel = g["mel"].astype(np.float32)        # [B, T, 80]
    f0 = g["f0"].astype(np.float32)          # [B, T, 4]

    # conv weights -> lhsT layout [ci, k, co]
    wc0 = np.ascontiguousarray(g["cont_w0"].transpose(1, 2, 0)).astype(np.float32)  # [80,5,512]
    wp0 = np.ascontiguousarray(g["pit_w0"].transpose(1, 2, 0)).astype(np.float32)   # [4,5,128]
    # layers 1,2: [l, ci_tile, 128, 5, co]
    wc12 = np.ascontiguousarray(
        g["cont_w"].transpose(0, 2, 3, 1).reshape(2, 4, 128, 5, CONT_H)).astype(np.float32)
    wp12 = np.ascontiguousarray(
        g["pit_w"].transpose(0, 2, 3, 1).reshape(2, 1, 128, 5, PIT_H)).astype(np.float32)

    # per-layer packs [L, 128, 5]: ctiles 0..3 = cont co slices, 4 = pit
    def pack(cont, pit):  # cont [L,512], pit [L,128] -> [L,128,5]
        out = np.empty((LAYERS, 128, 5), dtype=np.float32)
        for l in range(LAYERS):
            for j in range(4):
                out[l, :, j] = cont[l, j * 128:(j + 1) * 128]
            out[l, :, 4] = pit[l]
        return out
    bias_pack = pack(np.asarray(g["cont_b"], np.float32), np.asarray(g["pit_b"], np.float32))
    gamma_pack = pack(np.asarray(g["cont_gamma"], np.float32), np.asarray(g["pit_gamma"], np.float32))
    beta_pack = pack(np.asarray(g["cont_beta"], np.float32), np.asarray(g["pit_beta"], np.float32))

    scales = np.asarray(g["scales"], np.float32).reshape(LAYERS, B, NUM_SEG)
    len_seg = np.asarray(g["len_seg"]).reshape(LAYERS, B, NUM_SEG)

    A = [[_resample_matrix(scales[l, b], len_seg[l, b]) for b in range(B)]
         for l in range(LAYERS)]

    # static band tables (global over all B -> identical program on all cores)
    LO = np.zeros((2, NCHUNK), np.int64)
    KT = np.zeros((2, NCHUNK), np.int64)
    for l in range(2):
        for c in range(NCHUNK):
            lo, hi = _band(A[l], c * CHUNK, (c + 1) * CHUNK)
            LO[l, c], KT[l, c] = lo, hi - lo
    LO2 = np.zeros(NT, np.int64)
    KT2 = np.zeros(NT, np.int64)
    for t in range(NT):
        lo, hi = _band(A[2], t * 128, (t + 1) * 128)
        LO2[t], KT2[t] = lo, hi - lo

    # pack banded A blocks per core
    per_core = []
    for core in range(NCORES):
        bs = range(core * BPC, (core + 1) * BPC)
        m = {}
        # mel/f0 channel-major, time-padded by 2 on each side
        mel_cm = np.zeros((BPC, CONT_DIM, TPAD), np.float32)
        f0_cm = np.zeros((BPC, PIT_DIM, TPAD), np.float32)
        for i, b in enumerate(bs):
            mel_cm[i, :, 2:2 + T] = mel[b].T
            f0_cm[i, :, 2:2 + T] = f0[b].T
        m["mel"] = mel_cm
        m["f0"] = f0_cm
        m["wc0"] = wc0
        m["wp0"] = wp0
        m["wc12"] = wc12
        m["wp12"] = wp12
        m["bias_pack"] = bias_pack
        m["gamma_pack"] = gamma_pack
        m["beta_pack"] = beta_pack
        m["blockones"] = _blockones()
        m["zpad"] = np.zeros((128, 2), np.float32)
        m["ident"] = np.eye(128, dtype=np.float32)
        for l in range(2):
            for c in range(NCHUNK):
                kt, lo = int(KT[l, c]), int(LO[l, c])
                blk = np.zeros((BPC, kt, 128, CHUNK), np.float32)
                for i, b in enumerate(bs):
                    blk[i] = A[l][b][lo * 128:(lo + kt) * 128,
                                     c * CHUNK:(c + 1) * CHUNK].reshape(kt, 128, CHUNK)
                m[f"ra{l}c{c}"] = blk
        for t in range(NT):
            kt, lo = int(KT2[t]), int(LO2[t])
            blk = np.zeros((BPC, kt, 128, 128), np.float32)
            for i, b in enumerate(bs):
                blk[i] = A[2][b][lo * 128:(lo + kt) * 128,
                                 t * 128:(t + 1) * 128].reshape(kt, 128, 128)
            m[f"ra2t{t}"] = blk
        per_core.append(m)

    tables = dict(LO=LO, KT=KT, LO2=LO2, KT2=KT2)
    return per_core, tables


# ----------------------------------------------------------------------------
# Device program
# ----------------------------------------------------------------------------
def build_program(tables):
    LO, KT, LO2, KT2 = tables["LO"], tables["KT"], tables["LO2"], tables["KT2"]
    KTMAXW = max(int(KT.max()) * CHUNK, int(KT2.max()) * 128)  # A tile free width

    nc = bacc_mod.Bacc("TRN2", target_bir_lowering=False)
    AF = mybir.ActivationFunctionType

    mel_in = nc.dram_tensor("mel", [BPC, CONT_DIM, TPAD], R32, kind="ExternalInput")
    f0_in = nc.dram_tensor("f0", [BPC, PIT_DIM, TPAD], R32, kind="ExternalInput")
    wc0_in = nc.dram_tensor("wc0", [CONT_DIM, 5, CONT_H], R32, kind="ExternalInput")
    wp0_in = nc.dram_tensor("wp0", [PIT_DIM, 5, PIT_H], R32, kind="ExternalInput")
    wc12_in = nc.dram_tensor("wc12", [2, 4, 128, 5, CONT_H], R32, kind="ExternalInput")
    wp12_in = nc.dram_tensor("wp12", [2, 1, 128, 5, PIT_H], R32, kind="ExternalInput")
    bias_in = nc.dram_tensor("bias_pack", [LAYERS, 128, 5], F32, kind="ExternalInput")
    gamma_in = nc.dram_tensor("gamma_pack", [LAYERS, 128, 5], F32, kind="ExternalInput")
    beta_in = nc.dram_tensor("beta_pack", [LAYERS, 128, 5], F32, kind="ExternalInput")
    bones_in = nc.dram_tensor("blockones", [128, 128], F32, kind="ExternalInput")
    ident_in = nc.dram_tensor("ident", [128, 128], F32, kind="ExternalInput")
    ra = {}
    for l in range(2):
        for c in range(NCHUNK):
            ra[(l, c)] = nc.dram_tensor(f"ra{l}c{c}", [BPC, int(KT[l, c]), 128, CHUNK],
                                        R32, kind="ExternalInput")
    ra2 = {}
    for t in range(NT):
        ra2[t] = nc.dram_tensor(f"ra2t{t}", [BPC, int(KT2[t]), 128, 128],
                                R32, kind="ExternalInput")

    zpad_in = nc.dram_tensor("zpad", [128, 2], R32, kind="ExternalInput")
    mel_out = nc.dram_tensor("mel_out", [BPC, T, CONT_H], F32, kind="ExternalOutput")
    f0_out = nc.dram_tensor("f0_out", [BPC, T, PIT_H], F32, kind="ExternalOutput")

    with tile.TileContext(nc) as tc:
        with (
            tc.tile_pool(name="wpool", bufs=1) as wpool,
            tc.tile_pool(name="xpool", bufs=1) as xpool,
            tc.tile_pool(name="ypool", bufs=1) as ypool,
            tc.tile_pool(name="xtpool", bufs=1) as xtpool,
            tc.tile_pool(name="apool", bufs=2) as apool,
            tc.tile_pool(name="spool", bufs=2) as spool,
            tc.tile_pool(name="cpool", bufs=1) as cpool,
            tc.tile_pool(name="opool", bufs=4) as opool,
            tc.tile_pool(name="cvps", bufs=CFG["cv_bufs"], space="PSUM") as cvps,
            tc.tile_pool(name="rsps", bufs=CFG["rs_bufs"], space="PSUM") as rsps,
            tc.tile_pool(name="tpps", bufs=CFG["tp_bufs"], space="PSUM") as tpps,
        ):
            # ---- constants ----
            bones = cpool.tile([128, 128], F32, tag="bones")
            nc.sync.dma_start(bones, bones_in[:, :])
            ident = cpool.tile([128, 128], F32, tag="ident")
            nc.sync.dma_start(ident, ident_in[:, :])
            ident_r = cpool.tile([128, 128], R32, tag="identr")
            nc.vector.tensor_copy(ident_r, ident[:, :])
            eps_t = cpool.tile([128, 1], F32, tag="eps")
            nc.vector.memset(eps_t, EPS)

            # ---- persistent x tiles (conv inputs), zeroed borders ----
            xt_x = {}
            for b in range(BPC):
                for j in range(5):
                    xx = xpool.tile([128, TPAD], R32, tag=f"x_{b}_{j}")
                    xt_x[(b, j)] = xx
            # layer-0 inputs land in x_{b}_0 (mel, 80 parts), x_{b}_1 (f0, 4 parts)
            for b in range(BPC):
                nc.sync.dma_start(xt_x[(b, 0)][:CONT_DIM, :], mel_in[b])
                nc.sync.dma_start(xt_x[(b, 1)][:PIT_DIM, :], f0_in[b])
            # pad-border zeroing (needed from layer-1 on; keep off critical path)
            for b in range(BPC):
                for j in range(5):
                    xx = xt_x[(b, j)]
                    nc.gpsimd.dma_start(xx[:, 0:2], zpad_in[:, :])
                    nc.gpsimd.dma_start(xx[:, T + 2:TPAD], zpad_in[:, :])

            for l in range(LAYERS):
                # ---- load weights for this layer ----
                w_cont = []
                if l == 0:
                    w0 = wpool.tile([128, 5, CONT_H], R32, tag="wc0")
                    nc.sync.dma_start(w0[:CONT_DIM], wc0_in[:, :, :])
                    w_cont.append(w0)
                    wp = wpool.tile([128, 5, PIT_H], R32, tag="wp")
                    nc.sync.dma_start(wp[:PIT_DIM], wp0_in[:, :, :])
                else:
                    for ci_t in range(4):
                        wt = wpool.tile([128, 5, CONT_H], R32, tag=f"wc{ci_t}")
                        nc.sync.dma_start(wt, wc12_in[l - 1, ci_t])
                        w_cont.append(wt)
                    wp = wpool.tile([128, 5, PIT_H], R32, tag="wp")
                    nc.sync.dma_start(wp, wp12_in[l - 1, 0])
                gam = cpool.tile([128, 5], F32, tag="gam")
                nc.sync.dma_start(gam, gamma_in[l])
                bet = cpool.tile([128, 5], F32, tag="bet")
                nc.sync.dma_start(bet, beta_in[l])
                bia = cpool.tile([128, 5], F32, tag="bia")
                nc.sync.dma_start(bia, bias_in[l])

                # conv input tiles per stream
                def conv_srcs(b):
                    if l == 0:
                        return ([(xt_x[(b, 0)], CONT_DIM)], [(xt_x[(b, 1)], PIT_DIM)])
                    return ([(xt_x[(b, j)], 128) for j in range(4)],
                            [(xt_x[(b, 4)], 128)])

                for b in range(BPC):
                    cont_src, pit_src = conv_srcs(b)

                    # prefetch A blocks for this (l, b)
                    a_sb = {}
                    if l < 2:
                        for c in range(NCHUNK):
                            kt = int(KT[l, c])
                            at = apool.tile([128, KTMAXW], R32, tag=f"a{c % 2}")
                            nc.sync.dma_start(
                                at[:, 0:kt * CHUNK].rearrange("p (k n) -> p k n", k=kt),
                                ra[(l, c)][b].rearrange("k p n -> p k n"))
                            a_sb[c] = at
                    else:
                        for t in range(NT):
                            kt = int(KT2[t])
                            at = apool.tile([128, KTMAXW], R32, tag=f"a{t % 2}")
                            nc.sync.dma_start(
                                at[:, 0:kt * 128].rearrange("p (k n) -> p k n", k=kt),
                                ra2[t][b].rearrange("k p n -> p k n"))
                            a_sb[t] = at

                    # ---- conv + GN + ReLU per ctile ----
                    y_tiles = [None] * 5
                    for j in CFG.get("j_order", [0, 1, 2, 3, 4]):
                        is_pit = (j == 4)
                        srcs = pit_src if is_pit else cont_src
                        wlist = [wp] if is_pit else w_cont
                        co0 = 0 if is_pit else j * 128
                        if CFG["cv_shape"] == 1024:
                            pst = cvps.tile([128, 1024], F32, tag="cv")
                            halves = [pst[:, 0:512], pst[:, 512:1024]]
                        else:
                            halves = [cvps.tile([128, 512], F32, tag="cv",
                                                 name=f"cvh{b}_{j}_{h}")
                                      for h in range(2)]
                        nmm = len(srcs) * 5
                        st6 = spool.tile([128, 2, 6], F32, tag="st6")
                        if CFG.get("conv_pair", True):
                            i = 0
                            for si, (xsrc, kdim) in enumerate(srcs):
                                wt = wlist[si if not is_pit else 0] if not is_pit else wp
                                for k in range(5):
                                    for half in range(2):
                                        nc.tensor.matmul(
                                            halves[half],
                                            wt[:kdim, k, co0:co0 + 128],
                                            xsrc[:kdim, half * 512 + k:half * 512 + k + 512],
                                            start=(i == 0), stop=(i == nmm - 1))
                                    i += 1
                            for half in range(2):
                                nc.vector.bn_stats(st6[:, half, :], halves[half])
                        else:
                            for half in range(2):
                                i = 0
                                for si, (xsrc, kdim) in enumerate(srcs):
                                    wt = wlist[si if not is_pit else 0] if not is_pit else wp
                                    for k in range(5):
                                        nc.tensor.matmul(
                                            halves[half],
                                            wt[:kdim, k, co0:co0 + 128],
                                            xsrc[:kdim, half * 512 + k:half * 512 + k + 512],
                                            start=(i == 0), stop=(i == nmm - 1))
                                        i += 1
                                nc.vector.bn_stats(st6[:, half, :], halves[half])
                        mv = spool.tile([128, 3], F32, tag="mv")
                        nc.vector.bn_aggr(mv[:, 0:2], st6)
                        # mean' = mean + bias ; stash mean'^2
                        nc.vector.tensor_add(mv[:, 0:1], mv[:, 0:1], bia[:, j:j + 1])
                        nc.vector.tensor_mul(mv[:, 2:3], mv[:, 0:1], mv[:, 0:1])
                        # group reduce+broadcast: [mean', var, mean'^2] x blockones/16
                        gps = tpps.tile([128, 3], F32, tag="tp")
                        nc.tensor.matmul(gps, bones[:, :], mv[:, :], start=True, stop=True)
                        gs = spool.tile([128, 3], F32, tag="gs")
                        nc.vector.tensor_copy(gs, gps[:, 0:3])
                        # var_g = Ev + Em2 - mu^2 ; scale = gamma/sqrt(var+eps)
                        t1 = spool.tile([128, 1], F32, tag="t1")
                        nc.vector.tensor_mul(t1, gs[:, 0:1], gs[:, 0:1])
                        t2 = spool.tile([128, 1], F32, tag="t2")
                        nc.vector.tensor_add(t2, gs[:, 1:2], gs[:, 2:3])
                        nc.vector.tensor_sub(t2, t2, t1)
                        nc.scalar.activation(t2, t2, AF.Sqrt, bias=eps_t[:, 0:1])
                        nc.vector.reciprocal(t2, t2)
                        scl = spool.tile([128, 1], F32, tag="scl")
                        nc.vector.tensor_mul(scl, t2, gam[:, j:j + 1])
                        # bias_eff = (bias - mu_g)*scale + beta
                        bef = spool.tile([128, 1], F32, tag="bef")
                        nc.vector.tensor_sub(bef, bia[:, j:j + 1], gs[:, 0:1])
                        nc.vector.tensor_mul(bef, bef, scl)
                        nc.vector.tensor_add(bef, bef, bet[:, j:j + 1])
                        # apply + relu, PSUM -> SBUF (rounded to fp32r)
                        yt = ypool.tile([128, T], R32, tag=f"y{j}")
                        for half in range(2):
                            nc.scalar.activation(yt[:, half * 512:half * 512 + 512],
                                                 halves[half], AF.Relu,
                                                 bias=bef[:, 0:1], scale=scl[:, 0:1])
                        y_tiles[j] = yt

                    # ---- transpose y -> xT [128, t, 640] (one tensor per b) ----
                    xtall = xtpool.tile([128, NT, CH], R32, tag="xtall",
                                        name=f"xtall{b}")
                    for t in range(NT):
                        tp = tpps.tile([128, 512], F32, tag="tp",
                                       name=f"tpa{b}_{t}")
                        for j in range(4):
                            nc.tensor.transpose(
                                tp[:, j * 128:(j + 1) * 128].bitcast(R32),
                                y_tiles[j][:, t * 128:(t + 1) * 128],
                                ident_r[:, :])
                        nc.scalar.copy(xtall[:, t, 0:512], tp[:, :])
                    for tg in range(2):
                        tp2 = tpps.tile([128, 512], F32, tag="tp",
                                        name=f"tpb{b}_{tg}")
                        for dt_ in range(4):
                            nc.tensor.transpose(
                                tp2[:, dt_ * 128:(dt_ + 1) * 128].bitcast(R32),
                                y_tiles[4][:, (tg * 4 + dt_) * 128:(tg * 4 + dt_ + 1) * 128],
                                ident_r[:, :])
                        nc.vector.tensor_copy(
                            xtall[:, tg * 4:(tg + 1) * 4, 512:640],
                            tp2[:, :].rearrange("p (t n) -> p t n", t=4))
                    xt_t = [xtall[:, t, :] for t in range(NT)]

                    # ---- resample ----
                    if l < 2:
                        for j in range(5):
                            for hp in range(2):
                                ps = rsps.tile([128, 512], F32, tag="rs",
                                               name=f"rs{b}_{j}_{hp}")
                                for ci in range(2):
                                    c = hp * 2 + ci
                                    kt, lo = int(KT[l, c]), int(LO[l, c])
                                    for q in range(kt):
                                        nc.tensor.matmul(
                                            ps[:, ci * CHUNK:(ci + 1) * CHUNK],
                                            xt_t[lo + q][:, j * 128:(j + 1) * 128],
                                            a_sb[c][:, q * CHUNK:(q + 1) * CHUNK],
                                            start=(q == 0), stop=(q == kt - 1))
                                if CFG["rs_evict"] == "act":
                                    nc.scalar.copy(
                                        xt_x[(b, j)][:, 2 + hp * 512:2 + hp * 512 + 512],
                                        ps[:, 0:512])
                                else:
                                    nc.vector.tensor_copy(
                                        xt_x[(b, j)][:, 2 + hp * 512:2 + hp * 512 + 512],
                                        ps[:, 0:512])
                    else:
                        for t in range(NT):
                            kt, lo = int(KT2[t]), int(LO2[t])
                            psm = rsps.tile([128, 512], F32, tag="rs",
                                            name=f"rsm{b}_{t}")
                            for q in range(kt):
                                nc.tensor.matmul(
                                    psm,
                                    a_sb[t][:, q * 128:(q + 1) * 128],
                                    xt_t[lo + q][:, 0:512],
                                    start=(q == 0), stop=(q == kt - 1))
                            psf = tpps.tile([128, 512], F32, tag="tp",
                                            name=f"rsf{b}_{t}")
                            for q in range(kt):
                                nc.tensor.matmul(
                                    psf[:, 0:128],
                                    a_sb[t][:, q * 128:(q + 1) * 128],
                                    xt_t[lo + q][:, 512:640],
                                    start=(q == 0), stop=(q == kt - 1))
                            ost = opool.tile([128, CH], F32, tag="ost")
                            nc.scalar.copy(ost[:, 0:512], psm)
                            nc.vector.tensor_copy(ost[:, 512:640], psf[:, 0:128])
                            nc.gpsimd.dma_start(mel_out[b, t * 128:(t + 1) * 128, :],
                                                ost[:, 0:512])
                            nc.gpsimd.dma_start(f0_out[b, t * 128:(t + 1) * 128, :],
                                                ost[:, 512:640])
    nc.finalize()
    return nc


_CACHE = {}


def _get_program_and_inputs(inputs):
    per_core, tables = _prep_host(inputs)
    key = (tuple(tables["LO"].ravel()), tuple(tables["KT"].ravel()),
           tuple(tables["LO2"].ravel()), tuple(tables["KT2"].ravel()))
    if key not in _CACHE:
        _CACHE[key] = build_program(tables)
    return _CACHE[key], per_core


def kernel(**inputs):
    nc, per_core = _get_program_and_inputs(inputs)
    res = run_bass_kernel_spmd(nc, per_core, core_ids=list(range(NCORES)))
    mel = np.concatenate([r["mel_out"] for r in res.results], axis=0)
    f0 = np.concatenate([r["f0_out"] for r in res.results], axis=0)
    return mel, f0


def run_traced(inputs, **kw):
    """test.py helper: returns (results_object, per_core) for profiling."""
    nc, per_core = _get_program_and_inputs(inputs)
    return run_bass_kernel_spmd(nc, per_core, core_ids=list(range(NCORES)), **kw), per_core
